# revision 40
# baseline (speedup 1.0000x reference)
"""Trainium2 Bass kernel for nn_BinaryMemoryRNN (scatter_memory).

Computation (reference):
    logits = h_prev @ Mw.T + Mb                 # [B, 28]
    b1/b2  = bits of logits halves (> 0)
    idx1   = clip(sum(b1 * 2^(13-j)), 0, 8191)
    idx2   = clip(sum(b2 * 2^(13-j)), 8192, 16383)
    pre    = x @ Ww.T + h_prev @ Uw.T + mem[idx1] @ Qrw.T + mem[idx2] @ Qlw.T + bias
    out    = sigmoid(layernorm(pre) * gamma + beta)

Strategy: data-parallel over batch across 8 cores (1024 rows each).
  - x/h activations and all four weight matrices in fp8-e4m3 (weights
    scaled x256 to sit in e4m3's normal range; the layernorm that follows
    is scale-invariant so only the bias needs the same x256). fp8 without
    DoubleRow runs at bf16 speed but halves DMA and SBUF, and avoids the
    HAM activity throttle that DoubleRow triggers.
  - logits matmul in float32r (FP22): index bits are sign-sensitive, and
    fp32r keeps 13 mantissa bits at 4x the speed of true fp32.
  - memory table replicated in DRAM as bf16 [16384, 1024]; rows fetched
    with gpsimd.dma_gather(transpose=True), which lands them directly in
    [feature, batch] layout via the DMA transpose crossbar - no PE
    transposes, no PSUM bounce. The mem matmuls run bf16-lhsT x fp8-rhs.
  - bias enters the PSUM accumulation as a rank-1 (ones x bias) matmul,
    so the epilogue reads layernorm stats straight from PSUM.
  - rstd = 1/sqrt(var+eps) via DVE quake-seed + 2 Newton steps: the ACT
    engine only ever runs Sigmoid, avoiding per-tile activation-table
    reloads. Output written as bf16.
"""

import sys

sys.path.insert(0, "/opt/trn_rl_repo")

from contextlib import ExitStack

import numpy as np
import ml_dtypes

import concourse.bass as bass
import concourse.tile as tile
from concourse import bacc, mybir, library_config
from concourse.bass_utils import run_bass_kernel_spmd

F32 = mybir.dt.float32
F32R = mybir.dt.float32r
BF16 = mybir.dt.bfloat16
F8E4 = mybir.dt.float8e4
I16 = mybir.dt.int16
I32 = mybir.dt.int32

B, I, H, NB = 8192, 1024, 1024, 14
MEM = 2**NB
NCORES = 8
BL = B // NCORES  # 1024 batch rows per core
KC = H // 128  # 8 contraction chunks
MT = BL // 128  # 8 output row-tiles per core
EPS = 1e-5
WSCALE = 256.0
EPS_SC = EPS * WSCALE * WSCALE  # eps for the x256-scaled pre-activation
RSQRT_MAGIC = 0x5EF759DF  # 0x5f3759df - 0x00400000: seed for rsqrt(2*vh)

LOGITS_F32R = False  # fp32r logits (4x faster); False -> exact fp32

# const_t packed layout (f32 columns)
C_CLIP = 0  # [2, 2] idx clip bounds
C_NEGMB = 2  # [28, 1] -Mb
C_PW = 3  # [28, 1] powers of two as bf16 pair-packed in f32
C_MAGIC = 4  # [128, 1] rsqrt seed magic (int32 bits)
C_IDENT = 5  # [128, 32] 128x128 fp8 identity (bitcast)
NCONST = 37

_CACHE = {}


def _build(trivial_gb: bool):
    """Trace the Bass/Tile module (shared by all 8 cores, SPMD)."""
    nc = bacc.Bacc(
        "TRN2", target_bir_lowering=False, debug=False, enable_asserts=True
    )

    ldt = F32R if LOGITS_F32R else F32
    x8_t = nc.dram_tensor("x8_t", [128, KC, BL], F8E4, kind="ExternalInput").ap()
    h8_t = nc.dram_tensor("h8_t", [128, KC, BL], F8E4, kind="ExternalInput").ap()
    h32_t = nc.dram_tensor("h32_t", [128, KC, BL], ldt, kind="ExternalInput").ap()
    mw_t = nc.dram_tensor("mw_t", [128, KC, 2 * NB], ldt, kind="ExternalInput").ap()
    # weights, [src, feat_in(part), feat_in(chunk), feat_out]; src order W,U,Qr,Ql
    w8_t = nc.dram_tensor("w8_t", [4, 128, KC, H], F8E4, kind="ExternalInput").ap()
    bias8_t = nc.dram_tensor("bias8_t", [1, H], F8E4, kind="ExternalInput").ap()
    const_t = nc.dram_tensor("const_t", [128, NCONST], F32, kind="ExternalInput").ap()
    mem_t = nc.dram_tensor("mem_t", [MEM, H], F8E4, kind="ExternalInput").ap()
    if not trivial_gb:
        gam_t = nc.dram_tensor("gam_t", [128, H], F32, kind="ExternalInput").ap()
        bet_t = nc.dram_tensor("bet_t", [128, H], F32, kind="ExternalInput").ap()
    out_t = nc.dram_tensor("out_t", [BL, H], BF16, kind="ExternalOutput").ap()

    with tile.TileContext(nc) as tc:
        with ExitStack() as ctx:
            # ---------------- pools ----------------
            cpool = ctx.enter_context(tc.tile_pool(name="consts", bufs=1))
            apool = ctx.enter_context(tc.tile_pool(name="acts", bufs=1))
            hpool = ctx.enter_context(tc.tile_pool(name="h32mid", bufs=3))
            gpool = ctx.enter_context(tc.tile_pool(name="gathered", bufs=1))
            spool = ctx.enter_context(tc.tile_pool(name="small", bufs=2))
            epool = ctx.enter_context(tc.tile_pool(name="epilogue", bufs=2))
            pp_main = ctx.enter_context(
                tc.tile_pool(name="psum_main", bufs=2, space="PSUM")
            )
            # logits / idx / PE-transpose outputs share two 2-bank slots
            pp_small = ctx.enter_context(
                tc.tile_pool(name="psum_small", bufs=2, space="PSUM")
            )

            # gpsimd ucode library containing DMAGatherAnt; load it up front
            # so the Q7 IRAM reload overlaps the initial DMAs.
            nc.gpsimd.load_library(library_config.attnmlp)

            # ---------------- input loads ----------------
            # critical path first: packed consts + h fp32 for the index pipeline
            const_sb = cpool.tile([128, NCONST], F32, tag="const")
            nc.sync.dma_start(const_sb[:], const_t[:])
            mw_sb = cpool.tile([128, KC, 2 * NB], ldt, tag="mw")
            nc.sync.dma_start(mw_sb[:], mw_t[:])
            clip_sb = const_sb[0:2, C_CLIP : C_CLIP + 2]
            negmb_sb = const_sb[0 : 2 * NB, C_NEGMB : C_NEGMB + 1]
            pw_sb = const_sb[0 : 2 * NB, C_PW : C_PW + 1].bitcast(BF16)
            magic_sb = const_sb[:, C_MAGIC : C_MAGIC + 1].bitcast(I32)
            ident_sb = const_sb[:, C_IDENT : C_IDENT + 32].bitcast(F8E4)

            # warm the Sigmoid activation table while DMAs run (the only
            # ACT function in this kernel -> one table load total)
            warm = cpool.tile([128, 1], F32, tag="warm")
            nc.vector.memset(warm[:], 0.0)
            nc.scalar.activation(
                warm[:], warm[:], mybir.ActivationFunctionType.Sigmoid
            )

            ones_sb = cpool.tile([1, 128], F8E4, tag="ones")
            nc.vector.memset(ones_sb[:], 1.0)
            bias8_sb = cpool.tile([1, H], F8E4, tag="bias8")
            nc.sync.dma_start(bias8_sb[:], bias8_t[:])

            # h32 split 1/2/2/2/1 chunks: the logits matmul starts after the
            # first 512KB
            h32_k0 = spool.tile([128, 1, BL], ldt, tag="h32k0")
            nc.sync.dma_start(h32_k0[:], h32_t[:, 0:1, :])
            h32_mid = []
            for piece in range(3):
                hp = hpool.tile([128, 2, BL], ldt, tag="slab")
                nc.sync.dma_start(
                    hp[:], h32_t[:, 1 + 2 * piece : 3 + 2 * piece, :]
                )
                h32_mid.append(hp)
            h32_k7 = spool.tile([128, 1, BL], ldt, tag="h32k7")
            nc.sync.dma_start(h32_k7[:], h32_t[:, KC - 1 : KC, :])

            def h32_chunk(k):
                if k == 0:
                    return h32_k0[:, 0, :]
                if k == KC - 1:
                    return h32_k7[:, 0, :]
                return h32_mid[(k - 1) // 2][:, (k - 1) % 2, :]

            x8_sb = apool.tile([128, KC, BL], F8E4, tag="x8")
            nc.sync.dma_start(x8_sb[:], x8_t[:])
            h8_sb = apool.tile([128, KC, BL], F8E4, tag="h8")
            nc.sync.dma_start(h8_sb[:], h8_t[:])
            w_sb = []
            for s in range(4):
                w = cpool.tile([128, KC, H], F8E4, tag=f"w{s}")
                nc.sync.dma_start(w[:], w8_t[s])
                w_sb.append(w)
            if not trivial_gb:
                gam_sb = cpool.tile([128, H], F32, tag="gam")
                nc.sync.dma_start(gam_sb[:], gam_t[:])
                bet_sb = cpool.tile([128, H], F32, tag="bet")
                nc.sync.dma_start(bet_sb[:], bet_t[:])
                zero_sb = cpool.tile([128, 1], F32, tag="zero")
                nc.vector.memset(zero_sb[:], 0.0)

            # ---------------- index pipeline ----------------
            # logits.T [28, BL], accumulated over KC chunks; k-outer so the
            # first chunks of h32 are enough to start. fp32r = FP22 matmul
            # at 1 cycle/row (vs 4 for true fp32).
            logit_ps = pp_small.tile([2 * NB, BL], F32, tag="sm")
            for k in range(KC):
                hk = h32_chunk(k)
                for n in range(BL // 512):
                    nc.tensor.matmul(
                        logit_ps[:, n * 512 : (n + 1) * 512],
                        mw_sb[:, k, :],
                        hk[:, n * 512 : (n + 1) * 512],
                        start=(k == 0),
                        stop=(k == KC - 1),
                    )
            # bits = (h@Mw.T + Mb > 0)  <=>  (h@Mw.T > -Mb), as 1.0/0.0
            bits_sb = spool.tile([2 * NB, BL], BF16, tag="bits")
            nc.vector.tensor_scalar(
                bits_sb[:], logit_ps[:], negmb_sb[:, 0:1], None,
                mybir.AluOpType.is_gt,
            )
            # raw indices via tiny matmul with powers of two: [2, BL]
            idx_ps = pp_small.tile([2, BL], F32, tag="sm")
            for n in range(BL // 512):
                nc.tensor.matmul(
                    idx_ps[:, n * 512 : (n + 1) * 512],
                    pw_sb,
                    bits_sb[:, n * 512 : (n + 1) * 512],
                    start=True,
                    stop=True,
                )
            # clip + cast to int16; per-partition clip bounds:
            # row0 -> [0, 8191], row1 -> [8192, 16383]
            idx16 = spool.tile([2, BL], I16, tag="idx16")
            nc.vector.tensor_scalar(
                idx16[:], idx_ps[:], clip_sb[:, 0:1], clip_sb[:, 1:2],
                mybir.AluOpType.max, mybir.AluOpType.min,
            )

            # Wrap each index row into the [16, BL/16] layout dma_gather wants,
            # replicated to every 16-partition group (the Q7 ucode cores each
            # read their own group). Stage S[i, 32j+q'] = idx[(32j+i)*16+q'%16]
            # (16 columns duplicated within each 32-block), then four DVE
            # 32x32 block-transposes to partition bases 0/32/64/96.
            idxw_r = []
            for r in range(2):
                # issue on ACT's HWDGE FIFO so this tiny latency-critical
                # transfer doesn't queue behind the big input loads on SP's
                stg = spool.tile([32, 64], I16, tag="stage")
                stg_j = stg[0:32, :].rearrange("p (j hq) -> p j hq", j=2)
                with nc.allow_non_contiguous_dma(reason="tiny idx wrap staging"):
                    for j in range(2):
                        nc.scalar.dma_start(
                            stg[0:32, 32 * j : 32 * j + 16],
                            idx16[r : r + 1, j * 512 : (j + 1) * 512].rearrange(
                                "p (a b) -> p a b", b=16
                            ),
                        )
                nc.vector.tensor_copy(stg_j[:, :, 16:32], stg_j[:, :, 0:16])
                idxw = spool.tile([128, 64], I16, tag="idxw")
                for g in range(4):
                    nc.vector.transpose(idxw[32 * g : 32 * (g + 1), :], stg[:])
                idxw_r.append(idxw)

            # gathers split in batch halves, interleaved r0/r1, so blocks
            # c=0-3 of BOTH tensors arrive after the first two half-gathers.
            # g2[r][hf][p, c, :] = mem[idx_{(4*hf+c)*128+p}, :]  (fp8 rows)
            HB = BL // 2
            g2_tiles = [[None, None], [None, None]]
            for hf in range(2):
                for r in range(2):
                    g2 = gpool.tile([128, HB // 128, H], F8E4, tag=f"g2_{r}{hf}")
                    nc.gpsimd.dma_gather(
                        out_ap=g2[:],
                        in_ap=mem_t[:],
                        idxs_ap=idxw_r[r][:, hf * 32 : (hf + 1) * 32],
                        num_idxs=HB,
                        num_idxs_reg=HB,
                        elem_size=H,
                        transpose=False,
                    )
                    g2_tiles[r][hf] = g2

            # ---------------- main matmuls + epilogue ----------------
            srcs_xh = [(x8_sb, 0), (h8_sb, 1)]
            ps_tiles = {}
            mem_sb = [[None] * MT, [None] * MT]

            def emit_transpose(c):
                # PE-transpose gathered fp8 rows of batch-block c into
                # [feat, batch] fp8 tiles. The fp8 transpose datapath works
                # in 16-bit lanes, so the output lands at element step 2;
                # the DVE copy compacts it.
                for r in range(2):
                    g2 = g2_tiles[r][c // 4]
                    cc = c % 4
                    mt = gpool.tile([128, KC, 128], F8E4, tag=f"mem{r}_{c}")
                    for k in range(KC):
                        tp = pp_small.tile([128, 256], F8E4, tag="sm")
                        tp_s = tp.rearrange("p (b j) -> p b j", j=2)[:, :, 0]
                        nc.tensor.transpose(
                            tp_s, g2[:, cc, k * 128 : (k + 1) * 128],
                            ident_sb[:],
                        )
                        nc.vector.tensor_copy(mt[:, k, :], tp_s)
                    mem_sb[r][c] = mt

            def emit_xh(m):
                ps = pp_main.tile([128, H], F32, tag="acc")
                ps_tiles[m] = ps
                ms = slice(m * 128, (m + 1) * 128)
                # rank-1 bias matmul opens the accumulation group
                for n in range(H // 512):
                    nc.tensor.matmul(
                        ps[:, n * 512 : (n + 1) * 512],
                        ones_sb[:],
                        bias8_sb[:, n * 512 : (n + 1) * 512],
                        start=True,
                        stop=False,
                    )
                for si, (act, wi) in enumerate(srcs_xh):
                    for k in range(KC):
                        lhs = act[:, k, ms]
                        for n in range(H // 512):
                            nc.tensor.matmul(
                                ps[:, n * 512 : (n + 1) * 512],
                                lhs,
                                w_sb[wi][:, k, n * 512 : (n + 1) * 512],
                                start=False,
                                stop=False,
                            )

            def emit_mem_epilogue(m):
                ps = ps_tiles.pop(m)
                ms = slice(m * 128, (m + 1) * 128)
                for si in range(2):
                    mt = mem_sb[si][m]  # [128, KC, 128] block for this m
                    for k in range(KC):
                        lhs = mt[:, k, :]
                        for n in range(H // 512):
                            nc.tensor.matmul(
                                ps[:, n * 512 : (n + 1) * 512],
                                lhs,
                                w_sb[2 + si][:, k, n * 512 : (n + 1) * 512],
                                start=False,
                                stop=(si == 1 and k == KC - 1),
                            )

                # layernorm stats straight from PSUM (bias already inside)
                st6 = epool.tile([128, 2, 6], F32, tag="st6")
                for a in range(2):
                    nc.vector.bn_stats(st6[:, a, :], ps[:, a * 512 : (a + 1) * 512])
                mv = epool.tile([128, 2], F32, tag="mv")
                nc.vector.bn_aggr(mv[:], st6.rearrange("p a b -> p (a b)"))
                # rstd = 1/sqrt(var+eps) entirely on DVE:
                # quake seed from vh=(var+eps)/2 bits, then 2 Newton steps
                # y <- y*(1.5 - vh*y^2).
                st = epool.tile([128, 4], F32, tag="rs")
                vh = st[:, 0:1]
                y = st[:, 1:2]
                a_ = st[:, 2:3]
                nmu = st[:, 3:4]
                nc.vector.tensor_scalar(
                    vh, mv[:, 1:2], 0.5, EPS_SC * 0.5,
                    mybir.AluOpType.mult, mybir.AluOpType.add,
                )
                nc.vector.tensor_scalar(
                    a_.bitcast(I32), vh.bitcast(I32), 1, None,
                    mybir.AluOpType.logical_shift_right,
                )
                nc.vector.tensor_tensor(
                    y.bitcast(I32), magic_sb[:], a_.bitcast(I32),
                    mybir.AluOpType.subtract,
                )
                for _ in range(2):
                    nc.vector.tensor_tensor(a_, y, y, mybir.AluOpType.mult)
                    nc.vector.tensor_tensor(a_, a_, vh, mybir.AluOpType.mult)
                    nc.vector.tensor_scalar(
                        a_, a_, 1.5, -1.0,
                        mybir.AluOpType.subtract, mybir.AluOpType.mult,
                    )
                    nc.vector.tensor_tensor(y, y, a_, mybir.AluOpType.mult)
                nc.vector.tensor_scalar(
                    nmu, mv[:, 0:1], y, -1.0,
                    mybir.AluOpType.mult, mybir.AluOpType.mult,
                )
                o = epool.tile([128, H], BF16, tag="o")
                if trivial_gb:
                    # out = sigmoid((t - mu) * rstd), read from PSUM
                    nc.scalar.activation(
                        o[:], ps[:], mybir.ActivationFunctionType.Sigmoid,
                        bias=nmu, scale=y,
                    )
                else:
                    xh = epool.tile([128, H], F32, tag="xh")
                    nc.scalar.activation(
                        xh[:], ps[:], mybir.ActivationFunctionType.Identity,
                        bias=nmu, scale=y,
                    )
                    nc.vector.tensor_tensor(
                        xh[:], xh[:], gam_sb[:], mybir.AluOpType.mult
                    )
                    nc.vector.tensor_tensor(
                        xh[:], xh[:], bet_sb[:], mybir.AluOpType.add
                    )
                    nc.scalar.activation(
                        o[:], xh[:], mybir.ActivationFunctionType.Sigmoid,
                        bias=zero_sb[:, 0:1],
                    )
                nc.sync.dma_start(out_t[ms, :], o[:])

            # 3 PSUM bufs -> 3 xh tiles of runway while the gathers fly
            emit_xh(0)
            emit_xh(1)
            for c in range(4):
                emit_transpose(c)
            emit_mem_epilogue(0)
            emit_xh(2)
            emit_mem_epilogue(1)
            emit_xh(3)
            for c in range(4, 8):
                emit_transpose(c)
            emit_mem_epilogue(2)
            for m in range(4, MT):
                emit_xh(m)
                emit_mem_epilogue(m - 1)
            emit_mem_epilogue(MT - 1)

    nc.compile()  # bacc register allocation / DCE
    return nc


def _to_kxp(a, dtype):
    """[batch, feat] -> [128, KC, batch] with feat = k*128 + p."""
    t = np.ascontiguousarray(a.T.reshape(KC, 128, -1).transpose(1, 0, 2))
    return t.astype(dtype)


def prep(inputs):
    """Host-side shard/layout prep. Returns (in_maps, trivial_gb)."""
    x = np.asarray(inputs["x"], np.float32)
    h = np.asarray(inputs["h_prev"], np.float32)
    memory = np.asarray(inputs["memory"], np.float32)
    gamma = np.asarray(inputs["gamma"], np.float32)
    beta = np.asarray(inputs["beta"], np.float32)
    trivial_gb = bool(np.all(gamma == 1.0) and np.all(beta == 0.0))

    bf = ml_dtypes.bfloat16
    e4 = ml_dtypes.float8_e4m3
    # W is [out, in]; the kernel wants w[p, k, n] = W[n, k*128+p], which is
    # exactly _to_kxp applied to W with (out, in) in the (batch, feat) slots.
    w_cat = np.stack(
        [
            _to_kxp(np.asarray(inputs[n], np.float32) * WSCALE, e4)
            for n in ("Ww", "Uw", "Qrw", "Qlw")
        ]
    )
    mw = _to_kxp(np.asarray(inputs["Mw"], np.float32), np.float32)  # [128, KC, 28]

    pw = np.zeros((2 * NB, 2), np.float32)
    pw[:NB, 0] = 2.0 ** np.arange(NB - 1, -1, -1)
    pw[NB:, 1] = 2.0 ** np.arange(NB - 1, -1, -1)
    clip = np.array(
        [[0.0, MEM // 2 - 1], [MEM // 2, MEM - 1]], np.float32
    )  # [row, (lo, hi)]

    mem16 = memory.astype(e4)
    bias8 = (
        (
            np.asarray(inputs["Wb"], np.float32)
            + np.asarray(inputs["Ub"], np.float32)
            + np.asarray(inputs["Qrb"], np.float32)
            + np.asarray(inputs["Qlb"], np.float32)
        )
        * WSCALE
    ).astype(e4).reshape(1, H)

    # pack the small constants into one [128, NCONST] f32 buffer
    const = np.zeros((128, NCONST), np.float32)
    const[:2, C_CLIP : C_CLIP + 2] = clip
    const[: 2 * NB, C_NEGMB : C_NEGMB + 1] = -np.asarray(
        inputs["Mb"], np.float32
    ).reshape(2 * NB, 1)
    const[: 2 * NB, C_PW : C_PW + 1] = pw.astype(bf).view(np.float32)
    const[:, C_MAGIC : C_MAGIC + 1] = (
        np.full((128, 1), RSQRT_MAGIC, np.int32).view(np.float32)
    )
    ident8 = np.eye(128, dtype=np.float32).astype(e4)
    const[:, C_IDENT : C_IDENT + 32] = ident8.view(np.float32)

    common = dict(
        w8_t=w_cat, bias8_t=bias8, const_t=const, mem_t=mem16, mw_t=mw
    )
    if not trivial_gb:
        common["gam_t"] = np.ascontiguousarray(np.broadcast_to(gamma, (128, H)))
        common["bet_t"] = np.ascontiguousarray(np.broadcast_to(beta, (128, H)))

    in_maps = []
    for c in range(NCORES):
        xs = x[c * BL : (c + 1) * BL]
        hs = h[c * BL : (c + 1) * BL]
        in_maps.append(
            dict(
                x8_t=_to_kxp(xs, e4),
                h8_t=_to_kxp(hs, e4),
                h32_t=_to_kxp(hs, np.float32),
                **common,
            )
        )
    return in_maps, trivial_gb


def get_nc(trivial_gb):
    key = ("nc", trivial_gb)
    if key not in _CACHE:
        _CACHE[key] = _build(trivial_gb)
    return _CACHE[key]


def run(inputs, trace=False, **kw):
    in_maps, trivial_gb = prep(inputs)
    nc = get_nc(trivial_gb)
    res = run_bass_kernel_spmd(
        nc, in_maps, core_ids=list(range(NCORES)), trace=trace, **kw
    )
    out = np.concatenate([res.results[c]["out_t"] for c in range(NCORES)], axis=0)
    return out.astype(np.float32), res


def kernel(**inputs):
    return run(inputs)[0]


# revision 41
# speedup vs baseline: 1.1095x; 1.1095x over previous
"""Trainium2 Bass kernel for nn_BinaryMemoryRNN (scatter_memory).

Computation (reference):
    logits = h_prev @ Mw.T + Mb                 # [B, 28]
    b1/b2  = bits of logits halves (> 0)
    idx1   = clip(sum(b1 * 2^(13-j)), 0, 8191)
    idx2   = clip(sum(b2 * 2^(13-j)), 8192, 16383)
    pre    = x @ Ww.T + h_prev @ Uw.T + mem[idx1] @ Qrw.T + mem[idx2] @ Qlw.T + bias
    out    = sigmoid(layernorm(pre) * gamma + beta)

Strategy: data-parallel over batch across 8 cores (1024 rows each).
  - x/h activations and all four weight matrices in fp8-e4m3 (weights
    scaled x256 to sit in e4m3's normal range; the layernorm that follows
    is scale-invariant so only the bias needs the same x256). fp8 without
    DoubleRow runs at bf16 speed but halves DMA and SBUF, and avoids the
    HAM activity throttle that DoubleRow triggers.
  - logits matmul in float32r (FP22): index bits are sign-sensitive, and
    fp32r keeps 13 mantissa bits at 4x the speed of true fp32.
  - memory table replicated in DRAM as bf16 [16384, 1024]; rows fetched
    with gpsimd.dma_gather(transpose=True), which lands them directly in
    [feature, batch] layout via the DMA transpose crossbar - no PE
    transposes, no PSUM bounce. The mem matmuls run bf16-lhsT x fp8-rhs.
  - bias enters the PSUM accumulation as a rank-1 (ones x bias) matmul,
    so the epilogue reads layernorm stats straight from PSUM.
  - rstd = 1/sqrt(var+eps) via DVE quake-seed + 2 Newton steps: the ACT
    engine only ever runs Sigmoid, avoiding per-tile activation-table
    reloads. Output written as bf16.
"""

import sys

sys.path.insert(0, "/opt/trn_rl_repo")

from contextlib import ExitStack

import numpy as np
import ml_dtypes

import concourse.bass as bass
import concourse.tile as tile
from concourse import bacc, mybir, library_config
from concourse.bass_utils import run_bass_kernel_spmd

F32 = mybir.dt.float32
F32R = mybir.dt.float32r
BF16 = mybir.dt.bfloat16
F8E4 = mybir.dt.float8e4
I16 = mybir.dt.int16
I32 = mybir.dt.int32

B, I, H, NB = 8192, 1024, 1024, 14
MEM = 2**NB
NCORES = 8
BL = B // NCORES  # 1024 batch rows per core
KC = H // 128  # 8 contraction chunks
MT = BL // 128  # 8 output row-tiles per core
EPS = 1e-5
WSCALE = 256.0
EPS_SC = EPS * WSCALE * WSCALE  # eps for the x256-scaled pre-activation
RSQRT_MAGIC = 0x5EF759DF  # 0x5f3759df - 0x00400000: seed for rsqrt(2*vh)

LOGITS_F32R = True  # fp32r logits (4x faster); False -> exact fp32

# const_t packed layout (f32 columns)
C_CLIP = 0  # [2, 2] idx clip bounds
C_NEGMB = 2  # [28, 1] -Mb
C_PW = 3  # [28, 1] powers of two as bf16 pair-packed in f32
C_MAGIC = 4  # [128, 1] rsqrt seed magic (int32 bits)
C_IDENT = 5  # [128, 64] 128x128 bf16 identity (bitcast)
NCONST = 69

_CACHE = {}


def _build(trivial_gb: bool):
    """Trace the Bass/Tile module (shared by all 8 cores, SPMD)."""
    nc = bacc.Bacc(
        "TRN2", target_bir_lowering=False, debug=False, enable_asserts=True
    )

    ldt = F32R if LOGITS_F32R else F32
    x8_t = nc.dram_tensor("x8_t", [128, KC, BL], F8E4, kind="ExternalInput").ap()
    h8_t = nc.dram_tensor("h8_t", [128, KC, BL], F8E4, kind="ExternalInput").ap()
    h32_t = nc.dram_tensor("h32_t", [128, KC, BL], ldt, kind="ExternalInput").ap()
    mw_t = nc.dram_tensor("mw_t", [128, KC, 2 * NB], ldt, kind="ExternalInput").ap()
    # weights, [src, feat_in(part), feat_in(chunk), feat_out]; src order W,U,Qr,Ql
    w16_t = nc.dram_tensor("w16_t", [4, 128, KC, H], BF16, kind="ExternalInput").ap()
    bias16_t = nc.dram_tensor("bias16_t", [1, H], BF16, kind="ExternalInput").ap()
    const_t = nc.dram_tensor("const_t", [128, NCONST], F32, kind="ExternalInput").ap()
    mem_t = nc.dram_tensor("mem_t", [MEM, H], BF16, kind="ExternalInput").ap()
    if not trivial_gb:
        gam_t = nc.dram_tensor("gam_t", [128, H], F32, kind="ExternalInput").ap()
        bet_t = nc.dram_tensor("bet_t", [128, H], F32, kind="ExternalInput").ap()
    out_t = nc.dram_tensor("out_t", [BL, H], BF16, kind="ExternalOutput").ap()

    with tile.TileContext(nc) as tc:
        with ExitStack() as ctx:
            # ---------------- pools ----------------
            cpool = ctx.enter_context(tc.tile_pool(name="consts", bufs=1))
            apool = ctx.enter_context(tc.tile_pool(name="acts", bufs=1))
            hpool = ctx.enter_context(tc.tile_pool(name="h32mid", bufs=3))
            gpool = ctx.enter_context(tc.tile_pool(name="gathered", bufs=1))
            spool = ctx.enter_context(tc.tile_pool(name="small", bufs=2))
            epool = ctx.enter_context(tc.tile_pool(name="epilogue", bufs=2))
            pp_main = ctx.enter_context(
                tc.tile_pool(name="psum_main", bufs=2, space="PSUM")
            )
            # logits / idx / PE-transpose outputs share two 2-bank slots
            pp_small = ctx.enter_context(
                tc.tile_pool(name="psum_small", bufs=2, space="PSUM")
            )

            # gpsimd ucode library containing DMAGatherAnt; load it up front
            # so the Q7 IRAM reload overlaps the initial DMAs.
            nc.gpsimd.load_library(library_config.attnmlp)

            # ---------------- input loads ----------------
            # critical path first: packed consts + h fp32 for the index pipeline
            const_sb = cpool.tile([128, NCONST], F32, tag="const")
            nc.sync.dma_start(const_sb[:], const_t[:])
            mw_sb = cpool.tile([128, KC, 2 * NB], ldt, tag="mw")
            nc.sync.dma_start(mw_sb[:], mw_t[:])
            clip_sb = const_sb[0:2, C_CLIP : C_CLIP + 2]
            negmb_sb = const_sb[0 : 2 * NB, C_NEGMB : C_NEGMB + 1]
            pw_sb = const_sb[0 : 2 * NB, C_PW : C_PW + 1].bitcast(BF16)
            magic_sb = const_sb[:, C_MAGIC : C_MAGIC + 1].bitcast(I32)
            ident_sb = const_sb[:, C_IDENT : C_IDENT + 64].bitcast(BF16)

            # warm the Sigmoid activation table while DMAs run (the only
            # ACT function in this kernel -> one table load total)
            warm = cpool.tile([128, 1], F32, tag="warm")
            nc.vector.memset(warm[:], 0.0)
            nc.scalar.activation(
                warm[:], warm[:], mybir.ActivationFunctionType.Sigmoid
            )

            ones_sb = cpool.tile([1, 128], BF16, tag="ones")
            nc.vector.memset(ones_sb[:], 1.0)
            bias16_sb = cpool.tile([1, H], BF16, tag="bias16")
            nc.sync.dma_start(bias16_sb[:], bias16_t[:])

            # h32 split 1/2/2/2/1 chunks: the logits matmul starts after the
            # first 512KB
            h32_k0 = spool.tile([128, 1, BL], ldt, tag="h32k0")
            nc.sync.dma_start(h32_k0[:], h32_t[:, 0:1, :])
            h32_mid = []
            for piece in range(3):
                hp = hpool.tile([128, 2, BL], ldt, tag="slab")
                nc.sync.dma_start(
                    hp[:], h32_t[:, 1 + 2 * piece : 3 + 2 * piece, :]
                )
                h32_mid.append(hp)
            h32_k7 = spool.tile([128, 1, BL], ldt, tag="h32k7")
            nc.sync.dma_start(h32_k7[:], h32_t[:, KC - 1 : KC, :])

            def h32_chunk(k):
                if k == 0:
                    return h32_k0[:, 0, :]
                if k == KC - 1:
                    return h32_k7[:, 0, :]
                return h32_mid[(k - 1) // 2][:, (k - 1) % 2, :]

            x8_sb = apool.tile([128, KC, BL], F8E4, tag="x8")
            nc.sync.dma_start(x8_sb[:], x8_t[:])
            h8_sb = apool.tile([128, KC, BL], F8E4, tag="h8")
            nc.sync.dma_start(h8_sb[:], h8_t[:])
            w_sb = []
            for s in range(4):
                w = cpool.tile([128, KC, H], BF16, tag=f"w{s}")
                nc.sync.dma_start(w[:], w16_t[s])
                w_sb.append(w)
            if not trivial_gb:
                gam_sb = cpool.tile([128, H], F32, tag="gam")
                nc.sync.dma_start(gam_sb[:], gam_t[:])
                bet_sb = cpool.tile([128, H], F32, tag="bet")
                nc.sync.dma_start(bet_sb[:], bet_t[:])
                zero_sb = cpool.tile([128, 1], F32, tag="zero")
                nc.vector.memset(zero_sb[:], 0.0)

            # ---------------- index pipeline ----------------
            # logits.T [28, BL], accumulated over KC chunks; k-outer so the
            # first chunks of h32 are enough to start. fp32r = FP22 matmul
            # at 1 cycle/row (vs 4 for true fp32).
            logit_ps = pp_small.tile([2 * NB, BL], F32, tag="sm")
            for k in range(KC):
                hk = h32_chunk(k)
                for n in range(BL // 512):
                    nc.tensor.matmul(
                        logit_ps[:, n * 512 : (n + 1) * 512],
                        mw_sb[:, k, :],
                        hk[:, n * 512 : (n + 1) * 512],
                        start=(k == 0),
                        stop=(k == KC - 1),
                    )
            # bits = (h@Mw.T + Mb > 0)  <=>  (h@Mw.T > -Mb), as 1.0/0.0
            bits_sb = spool.tile([2 * NB, BL], BF16, tag="bits")
            nc.vector.tensor_scalar(
                bits_sb[:], logit_ps[:], negmb_sb[:, 0:1], None,
                mybir.AluOpType.is_gt,
            )
            # raw indices via tiny matmul with powers of two: [2, BL]
            idx_ps = pp_small.tile([2, BL], F32, tag="sm")
            for n in range(BL // 512):
                nc.tensor.matmul(
                    idx_ps[:, n * 512 : (n + 1) * 512],
                    pw_sb,
                    bits_sb[:, n * 512 : (n + 1) * 512],
                    start=True,
                    stop=True,
                )
            # clip + cast to int16; per-partition clip bounds:
            # row0 -> [0, 8191], row1 -> [8192, 16383]
            idx16 = spool.tile([2, BL], I16, tag="idx16")
            nc.vector.tensor_scalar(
                idx16[:], idx_ps[:], clip_sb[:, 0:1], clip_sb[:, 1:2],
                mybir.AluOpType.max, mybir.AluOpType.min,
            )

            # Wrap each index row into the [16, BL/16] layout dma_gather wants,
            # replicated to every 16-partition group (the Q7 ucode cores each
            # read their own group). Stage S[i, 32j+q'] = idx[(32j+i)*16+q'%16]
            # (16 columns duplicated within each 32-block), then four DVE
            # 32x32 block-transposes to partition bases 0/32/64/96.
            idxw_r = []
            for r in range(2):
                # issue on ACT's HWDGE FIFO so this tiny latency-critical
                # transfer doesn't queue behind the big input loads on SP's
                stg = spool.tile([32, 64], I16, tag="stage")
                stg_j = stg[0:32, :].rearrange("p (j hq) -> p j hq", j=2)
                with nc.allow_non_contiguous_dma(reason="tiny idx wrap staging"):
                    for j in range(2):
                        nc.scalar.dma_start(
                            stg[0:32, 32 * j : 32 * j + 16],
                            idx16[r : r + 1, j * 512 : (j + 1) * 512].rearrange(
                                "p (a b) -> p a b", b=16
                            ),
                        )
                nc.vector.tensor_copy(stg_j[:, :, 16:32], stg_j[:, :, 0:16])
                idxw = spool.tile([128, 64], I16, tag="idxw")
                for g in range(4):
                    nc.vector.transpose(idxw[32 * g : 32 * (g + 1), :], stg[:])
                idxw_r.append(idxw)

            # gathers split in batch halves, interleaved r0/r1, so blocks
            # c=0-3 of BOTH tensors arrive after the first two half-gathers.
            # g2[r][hf][p, c, :] = mem[idx_{(4*hf+c)*128+p}, :]  (fp8 rows)
            HB = BL // 2
            g2_tiles = [[None, None], [None, None]]
            for hf in range(2):
                for r in range(2):
                    g2 = gpool.tile([128, HB // 128, H], BF16, tag=f"g2_{r}{hf}")
                    nc.gpsimd.dma_gather(
                        out_ap=g2[:],
                        in_ap=mem_t[:],
                        idxs_ap=idxw_r[r][:, hf * 32 : (hf + 1) * 32],
                        num_idxs=HB,
                        num_idxs_reg=HB,
                        elem_size=H,
                        transpose=False,
                    )
                    g2_tiles[r][hf] = g2

            # ---------------- main matmuls + epilogue ----------------
            srcs_xh = [(x8_sb, 0), (h8_sb, 1)]
            ps_tiles = {}
            mem_sb = [[None] * MT, [None] * MT]

            def emit_transpose(c):
                # PE-transpose gathered fp8 rows of batch-block c into
                # [feat, batch] fp8 tiles. The fp8 transpose datapath works
                # in 16-bit lanes, so the output lands at element step 2;
                # the DVE copy compacts it.
                for r in range(2):
                    g2 = g2_tiles[r][c // 4]
                    cc = c % 4
                    mt = gpool.tile([128, KC, 128], BF16, tag=f"mem{r}_{c}")
                    for k in range(KC):
                        tp = pp_small.tile([128, 128], BF16, tag="sm")
                        nc.tensor.transpose(
                            tp[:], g2[:, cc, k * 128 : (k + 1) * 128],
                            ident_sb[:],
                        )
                        nc.vector.tensor_copy(mt[:, k, :], tp[:])
                    mem_sb[r][c] = mt

            def emit_xh(m):
                ps = pp_main.tile([128, H], F32, tag="acc")
                ps_tiles[m] = ps
                ms = slice(m * 128, (m + 1) * 128)
                # rank-1 bias matmul opens the accumulation group
                for n in range(H // 512):
                    nc.tensor.matmul(
                        ps[:, n * 512 : (n + 1) * 512],
                        ones_sb[:],
                        bias16_sb[:, n * 512 : (n + 1) * 512],
                        start=True,
                        stop=False,
                    )
                for si, (act, wi) in enumerate(srcs_xh):
                    for k in range(KC):
                        lhs = act[:, k, ms]
                        for n in range(H // 512):
                            nc.tensor.matmul(
                                ps[:, n * 512 : (n + 1) * 512],
                                lhs,
                                w_sb[wi][:, k, n * 512 : (n + 1) * 512],
                                start=False,
                                stop=False,
                            )

            def emit_mem_epilogue(m):
                ps = ps_tiles.pop(m)
                ms = slice(m * 128, (m + 1) * 128)
                for si in range(2):
                    mt = mem_sb[si][m]  # [128, KC, 128] block for this m
                    for k in range(KC):
                        lhs = mt[:, k, :]
                        for n in range(H // 512):
                            nc.tensor.matmul(
                                ps[:, n * 512 : (n + 1) * 512],
                                lhs,
                                w_sb[2 + si][:, k, n * 512 : (n + 1) * 512],
                                start=False,
                                stop=(si == 1 and k == KC - 1),
                            )

                # layernorm stats straight from PSUM (bias already inside)
                st6 = epool.tile([128, 2, 6], F32, tag="st6")
                for a in range(2):
                    nc.vector.bn_stats(st6[:, a, :], ps[:, a * 512 : (a + 1) * 512])
                mv = epool.tile([128, 2], F32, tag="mv")
                nc.vector.bn_aggr(mv[:], st6.rearrange("p a b -> p (a b)"))
                # rstd = 1/sqrt(var+eps) entirely on DVE:
                # quake seed from vh=(var+eps)/2 bits, then 2 Newton steps
                # y <- y*(1.5 - vh*y^2).
                st = epool.tile([128, 4], F32, tag="rs")
                vh = st[:, 0:1]
                y = st[:, 1:2]
                a_ = st[:, 2:3]
                nmu = st[:, 3:4]
                nc.vector.tensor_scalar(
                    vh, mv[:, 1:2], 0.5, EPS_SC * 0.5,
                    mybir.AluOpType.mult, mybir.AluOpType.add,
                )
                nc.vector.tensor_scalar(
                    a_.bitcast(I32), vh.bitcast(I32), 1, None,
                    mybir.AluOpType.logical_shift_right,
                )
                nc.vector.tensor_tensor(
                    y.bitcast(I32), magic_sb[:], a_.bitcast(I32),
                    mybir.AluOpType.subtract,
                )
                for _ in range(2):
                    nc.vector.tensor_tensor(a_, y, y, mybir.AluOpType.mult)
                    nc.vector.tensor_tensor(a_, a_, vh, mybir.AluOpType.mult)
                    nc.vector.tensor_scalar(
                        a_, a_, 1.5, -1.0,
                        mybir.AluOpType.subtract, mybir.AluOpType.mult,
                    )
                    nc.vector.tensor_tensor(y, y, a_, mybir.AluOpType.mult)
                nc.vector.tensor_scalar(
                    nmu, mv[:, 0:1], y, -1.0,
                    mybir.AluOpType.mult, mybir.AluOpType.mult,
                )
                o = epool.tile([128, H], BF16, tag="o")
                if trivial_gb:
                    # out = sigmoid((t - mu) * rstd), read from PSUM
                    nc.scalar.activation(
                        o[:], ps[:], mybir.ActivationFunctionType.Sigmoid,
                        bias=nmu, scale=y,
                    )
                else:
                    xh = epool.tile([128, H], F32, tag="xh")
                    nc.scalar.activation(
                        xh[:], ps[:], mybir.ActivationFunctionType.Identity,
                        bias=nmu, scale=y,
                    )
                    nc.vector.tensor_tensor(
                        xh[:], xh[:], gam_sb[:], mybir.AluOpType.mult
                    )
                    nc.vector.tensor_tensor(
                        xh[:], xh[:], bet_sb[:], mybir.AluOpType.add
                    )
                    nc.scalar.activation(
                        o[:], xh[:], mybir.ActivationFunctionType.Sigmoid,
                        bias=zero_sb[:, 0:1],
                    )
                nc.sync.dma_start(out_t[ms, :], o[:])

            # 3 PSUM bufs -> 3 xh tiles of runway while the gathers fly
            emit_xh(0)
            emit_xh(1)
            for c in range(4):
                emit_transpose(c)
            emit_mem_epilogue(0)
            emit_xh(2)
            emit_mem_epilogue(1)
            emit_xh(3)
            for c in range(4, 8):
                emit_transpose(c)
            emit_mem_epilogue(2)
            for m in range(4, MT):
                emit_xh(m)
                emit_mem_epilogue(m - 1)
            emit_mem_epilogue(MT - 1)

    nc.compile()  # bacc register allocation / DCE
    return nc


def _to_kxp(a, dtype):
    """[batch, feat] -> [128, KC, batch] with feat = k*128 + p."""
    t = np.ascontiguousarray(a.T.reshape(KC, 128, -1).transpose(1, 0, 2))
    return t.astype(dtype)


def prep(inputs):
    """Host-side shard/layout prep. Returns (in_maps, trivial_gb)."""
    x = np.asarray(inputs["x"], np.float32)
    h = np.asarray(inputs["h_prev"], np.float32)
    memory = np.asarray(inputs["memory"], np.float32)
    gamma = np.asarray(inputs["gamma"], np.float32)
    beta = np.asarray(inputs["beta"], np.float32)
    trivial_gb = bool(np.all(gamma == 1.0) and np.all(beta == 0.0))

    bf = ml_dtypes.bfloat16
    e4 = ml_dtypes.float8_e4m3
    # W is [out, in]; the kernel wants w[p, k, n] = W[n, k*128+p], which is
    # exactly _to_kxp applied to W with (out, in) in the (batch, feat) slots.
    w_cat = np.stack(
        [
            _to_kxp(np.asarray(inputs[n], np.float32) * WSCALE, bf)
            for n in ("Ww", "Uw", "Qrw", "Qlw")
        ]
    )
    mw = _to_kxp(np.asarray(inputs["Mw"], np.float32), np.float32)  # [128, KC, 28]

    pw = np.zeros((2 * NB, 2), np.float32)
    pw[:NB, 0] = 2.0 ** np.arange(NB - 1, -1, -1)
    pw[NB:, 1] = 2.0 ** np.arange(NB - 1, -1, -1)
    clip = np.array(
        [[0.0, MEM // 2 - 1], [MEM // 2, MEM - 1]], np.float32
    )  # [row, (lo, hi)]

    mem16 = memory.astype(bf)
    bias16 = (
        (
            np.asarray(inputs["Wb"], np.float32)
            + np.asarray(inputs["Ub"], np.float32)
            + np.asarray(inputs["Qrb"], np.float32)
            + np.asarray(inputs["Qlb"], np.float32)
        )
        * WSCALE
    ).astype(bf).reshape(1, H)

    # pack the small constants into one [128, NCONST] f32 buffer
    const = np.zeros((128, NCONST), np.float32)
    const[:2, C_CLIP : C_CLIP + 2] = clip
    const[: 2 * NB, C_NEGMB : C_NEGMB + 1] = -np.asarray(
        inputs["Mb"], np.float32
    ).reshape(2 * NB, 1)
    const[: 2 * NB, C_PW : C_PW + 1] = pw.astype(bf).view(np.float32)
    const[:, C_MAGIC : C_MAGIC + 1] = (
        np.full((128, 1), RSQRT_MAGIC, np.int32).view(np.float32)
    )
    ident16 = np.eye(128, dtype=np.float32).astype(bf)
    const[:, C_IDENT : C_IDENT + 64] = ident16.view(np.float32)

    common = dict(
        w16_t=w_cat, bias16_t=bias16, const_t=const, mem_t=mem16, mw_t=mw
    )
    if not trivial_gb:
        common["gam_t"] = np.ascontiguousarray(np.broadcast_to(gamma, (128, H)))
        common["bet_t"] = np.ascontiguousarray(np.broadcast_to(beta, (128, H)))

    in_maps = []
    for c in range(NCORES):
        xs = x[c * BL : (c + 1) * BL]
        hs = h[c * BL : (c + 1) * BL]
        in_maps.append(
            dict(
                x8_t=_to_kxp(xs, e4),
                h8_t=_to_kxp(hs, e4),
                h32_t=_to_kxp(hs, np.float32),
                **common,
            )
        )
    return in_maps, trivial_gb


def get_nc(trivial_gb):
    key = ("nc", trivial_gb)
    if key not in _CACHE:
        _CACHE[key] = _build(trivial_gb)
    return _CACHE[key]


def run(inputs, trace=False, **kw):
    in_maps, trivial_gb = prep(inputs)
    nc = get_nc(trivial_gb)
    res = run_bass_kernel_spmd(
        nc, in_maps, core_ids=list(range(NCORES)), trace=trace, **kw
    )
    out = np.concatenate([res.results[c]["out_t"] for c in range(NCORES)], axis=0)
    return out.astype(np.float32), res


def kernel(**inputs):
    return run(inputs)[0]


# revision 45
# speedup vs baseline: 1.1774x; 1.0612x over previous
"""Trainium2 Bass kernel for nn_BinaryMemoryRNN (scatter_memory).

Computation (reference):
    logits = h_prev @ Mw.T + Mb                 # [B, 28]
    b1/b2  = bits of logits halves (> 0)
    idx1   = clip(sum(b1 * 2^(13-j)), 0, 8191)
    idx2   = clip(sum(b2 * 2^(13-j)), 8192, 16383)
    pre    = x @ Ww.T + h_prev @ Uw.T + mem[idx1] @ Qrw.T + mem[idx2] @ Qlw.T + bias
    out    = sigmoid(layernorm(pre) * gamma + beta)

Strategy: data-parallel over batch across 8 cores (1024 rows each).
  - x/h activations in fp8-e4m3 as the stationary matmul operand; weights
    stream as bf16 (the fp8xfp8 combination runs at half rate on TRN2, and
    fp8 as the moving operand is slow - fp8-stationary x bf16-moving is
    full rate). Weights scaled x256 so fp8 activations meet them in a
    range the layernorm renormalizes away; only the bias needs the x256.
  - logits matmul in fp16 (sign-sensitive index bits: fp16's 10 mantissa
    bits flip ~1e-3 of them; full fp32 would cost 4x the PE time and 2x
    the critical-path DMA).
  - memory table replicated in DRAM as bf16 [16384, 1024]; rows fetched
    with gpsimd.dma_gather (row layout), PE-transposed to [feat, batch].
    PSUM->SBUF copies of transposed tiles alternate DVE / ACT engines.
  - bias enters the PSUM accumulation as a rank-1 (ones x bias) matmul,
    so the epilogue reads layernorm stats straight from PSUM; mem matmuls
    are emitted bank-outer so bank-0 stats overlap bank-1 matmuls.
  - rstd = 1/sqrt(var+eps) via DVE quake-seed + 2 Newton steps: the ACT
    engine's activation table never leaves Sigmoid. Output written bf16.
  - DMA issue order doubles as the priority schedule: logits path first
    (mw, h16 pieces), then x/w0 for the first matmuls, then the rest.
"""

import sys

sys.path.insert(0, "/opt/trn_rl_repo")

from contextlib import ExitStack

import numpy as np
import ml_dtypes

import concourse.bass as bass
import concourse.tile as tile
from concourse import bacc, mybir, library_config
from concourse.bass_utils import run_bass_kernel_spmd

F32 = mybir.dt.float32
F16 = mybir.dt.float16
BF16 = mybir.dt.bfloat16
F8E4 = mybir.dt.float8e4
I16 = mybir.dt.int16
I32 = mybir.dt.int32

B, I, H, NB = 8192, 1024, 1024, 14
MEM = 2**NB
NCORES = 8
BL = B // NCORES  # 1024 batch rows per core
KC = H // 128  # 8 contraction chunks
MT = BL // 128  # 8 output row-tiles per core
EPS = 1e-5
WSCALE = 256.0
EPS_SC = EPS * WSCALE * WSCALE  # eps for the x256-scaled pre-activation
RSQRT_MAGIC = 0x5EF759DF  # 0x5f3759df - 0x00400000: seed for rsqrt(2*vh)

# const_t packed layout (f32 columns)
C_CLIP = 0  # [2, 2] idx clip bounds
C_NEGMB = 2  # [28, 1] -Mb
C_PW = 3  # [28, 1] powers of two as bf16 pair-packed in f32
C_MAGIC = 4  # [128, 1] rsqrt seed magic (int32 bits)
C_IDENT = 5  # [128, 64] 128x128 bf16 identity (bitcast)
NCONST = 69

_CACHE = {}


def _build(trivial_gb: bool):
    """Trace the Bass/Tile module (shared by all 8 cores, SPMD)."""
    nc = bacc.Bacc(
        "TRN2", target_bir_lowering=False, debug=False, enable_asserts=True
    )

    x8_t = nc.dram_tensor("x8_t", [128, KC, BL], F8E4, kind="ExternalInput").ap()
    h8_t = nc.dram_tensor("h8_t", [128, KC, BL], F8E4, kind="ExternalInput").ap()
    h16_t = nc.dram_tensor("h16_t", [128, KC, BL], F16, kind="ExternalInput").ap()
    mw_t = nc.dram_tensor("mw_t", [128, KC, 2 * NB], F16, kind="ExternalInput").ap()
    # weights, [src, feat_in(part), feat_in(chunk), feat_out]; src order W,U,Qr,Ql
    w16_t = nc.dram_tensor("w16_t", [4, 128, KC, H], BF16, kind="ExternalInput").ap()
    bias16_t = nc.dram_tensor("bias16_t", [1, H], BF16, kind="ExternalInput").ap()
    const_t = nc.dram_tensor("const_t", [128, NCONST], F32, kind="ExternalInput").ap()
    mem_t = nc.dram_tensor("mem_t", [MEM, H], BF16, kind="ExternalInput").ap()
    if not trivial_gb:
        gam_t = nc.dram_tensor("gam_t", [128, H], F32, kind="ExternalInput").ap()
        bet_t = nc.dram_tensor("bet_t", [128, H], F32, kind="ExternalInput").ap()
    out_t = nc.dram_tensor("out_t", [BL, H], BF16, kind="ExternalOutput").ap()

    with tile.TileContext(nc) as tc:
        with ExitStack() as ctx:
            # ---------------- pools ----------------
            cpool = ctx.enter_context(tc.tile_pool(name="consts", bufs=1))
            apool = ctx.enter_context(tc.tile_pool(name="acts", bufs=1))
            hpool = ctx.enter_context(tc.tile_pool(name="h16", bufs=4))
            gpool = ctx.enter_context(tc.tile_pool(name="gathered", bufs=1))
            spool = ctx.enter_context(tc.tile_pool(name="small", bufs=2))
            epool = ctx.enter_context(tc.tile_pool(name="epilogue", bufs=2))
            # PSUM: tag "acc" rotates 3 two-bank slots (logits shares the
            # ring); tag "sm" rotates 2 one-bank slots (idx halves + PE
            # transposes). 3*2 + 2*1 = 8 banks.
            pp = ctx.enter_context(tc.tile_pool(name="psum", bufs=1, space="PSUM"))

            # gpsimd ucode library containing DMAGatherAnt; load it up front
            # so the Q7 IRAM reload overlaps the initial DMAs.
            nc.gpsimd.load_library(library_config.attnmlp)

            # ---------------- input loads (issue order = priority) ----------
            mw_sb = cpool.tile([128, KC, 2 * NB], F16, tag="mw")
            nc.sync.dma_start(mw_sb[:], mw_t[:])
            const_sb = cpool.tile([128, NCONST], F32, tag="const")
            nc.sync.dma_start(const_sb[:], const_t[:])
            clip_sb = const_sb[0:2, C_CLIP : C_CLIP + 2]
            negmb_sb = const_sb[0 : 2 * NB, C_NEGMB : C_NEGMB + 1]
            pw_sb = const_sb[0 : 2 * NB, C_PW : C_PW + 1].bitcast(BF16)
            magic_sb = const_sb[:, C_MAGIC : C_MAGIC + 1].bitcast(I32)
            ident_sb = const_sb[:, C_IDENT : C_IDENT + 64].bitcast(BF16)

            # warm the Sigmoid activation table while DMAs run (the only
            # ACT table in this kernel -> one table load total)
            warm = cpool.tile([128, 1], F32, tag="warm")
            nc.vector.memset(warm[:], 0.0)
            nc.scalar.activation(
                warm[:], warm[:], mybir.ActivationFunctionType.Sigmoid
            )
            ones_sb = cpool.tile([1, 128], BF16, tag="ones")
            nc.vector.memset(ones_sb[:], 1.0)

            # h16 for logits in 4 pieces, interleaved with x8/w0 so the
            # first xh matmuls aren't starved behind the full logits load
            h16p = []
            for piece in range(4):
                hp = hpool.tile([128, 2, BL], F16, tag="slab")
                h16p.append(hp)
            x8_sb = apool.tile([128, KC, BL], F8E4, tag="x8")
            h8_sb = apool.tile([128, KC, BL], F8E4, tag="h8")
            w_sb = [
                cpool.tile([128, KC, H], BF16, tag=f"w{s}", name=f"w{s}")
                for s in range(4)
            ]
            bias16_sb = cpool.tile([1, H], BF16, tag="bias16")

            nc.sync.dma_start(h16p[0][:], h16_t[:, 0:2, :])
            nc.sync.dma_start(h16p[1][:], h16_t[:, 2:4, :])
            nc.sync.dma_start(x8_sb[:], x8_t[:])
            nc.sync.dma_start(h16p[2][:], h16_t[:, 4:6, :])
            nc.sync.dma_start(w_sb[0][:], w16_t[0])
            nc.sync.dma_start(h16p[3][:], h16_t[:, 6:8, :])
            nc.sync.dma_start(h8_sb[:], h8_t[:])
            nc.sync.dma_start(w_sb[1][:], w16_t[1])
            nc.sync.dma_start(bias16_sb[:], bias16_t[:])
            nc.sync.dma_start(w_sb[2][:], w16_t[2])
            nc.sync.dma_start(w_sb[3][:], w16_t[3])
            if not trivial_gb:
                gam_sb = cpool.tile([128, H], F32, tag="gam")
                nc.sync.dma_start(gam_sb[:], gam_t[:])
                bet_sb = cpool.tile([128, H], F32, tag="bet")
                nc.sync.dma_start(bet_sb[:], bet_t[:])
                zero_sb = cpool.tile([128, 1], F32, tag="zero")
                nc.vector.memset(zero_sb[:], 0.0)

            def h16_chunk(k):
                return h16p[k // 2][:, k % 2, :]

            # ---------------- index pipeline ----------------
            # logits.T [28, BL] fp16 inputs, fp32 PSUM; k-outer so the
            # first pieces of h16 are enough to start
            logit_ps = pp.tile([2 * NB, BL], F32, tag="acc", bufs=3)
            for k in range(KC):
                hk = h16_chunk(k)
                for n in range(BL // 512):
                    nc.tensor.matmul(
                        logit_ps[:, n * 512 : (n + 1) * 512],
                        mw_sb[:, k, :],
                        hk[:, n * 512 : (n + 1) * 512],
                        start=(k == 0),
                        stop=(k == KC - 1),
                    )
            # bits = (h@Mw.T + Mb > 0)  <=>  (h@Mw.T > -Mb), as 1.0/0.0
            bits_sb = spool.tile([2 * NB, BL], BF16, tag="bits")
            nc.vector.tensor_scalar(
                bits_sb[:], logit_ps[:], negmb_sb[:, 0:1], None,
                mybir.AluOpType.is_gt,
            )
            # raw indices via tiny matmul with powers of two: 2x [2, 512]
            idx16 = spool.tile([2, BL], I16, tag="idx16")
            for n in range(BL // 512):
                idx_ps = pp.tile([2, 512], F32, tag="sm", bufs=2)
                nc.tensor.matmul(
                    idx_ps[:],
                    pw_sb,
                    bits_sb[:, n * 512 : (n + 1) * 512],
                    start=True,
                    stop=True,
                )
                # clip + cast to int16; per-partition clip bounds:
                # row0 -> [0, 8191], row1 -> [8192, 16383]
                nc.vector.tensor_scalar(
                    idx16[:, n * 512 : (n + 1) * 512], idx_ps[:],
                    clip_sb[:, 0:1], clip_sb[:, 1:2],
                    mybir.AluOpType.max, mybir.AluOpType.min,
                )

            # Wrap each index row into the [16, BL/16] layout dma_gather wants,
            # replicated to every 16-partition group (the Q7 ucode cores each
            # read their own group). Stage S[i, 32j+q'] = idx[(32j+i)*16+q'%16]
            # (16 columns duplicated within each 32-block), then four DVE
            # 32x32 block-transposes to partition bases 0/32/64/96.
            idxw_r = []
            for r in range(2):
                # one strided DMA per r on ACT's HWDGE FIFO (tiny,
                # latency-critical; SP's queue is busy with input loads)
                stg = spool.tile([32, 64], I16, tag="stage")
                stg_j = stg[0:32, :].rearrange("p (j hq) -> p j hq", j=2)
                with nc.allow_non_contiguous_dma(reason="tiny idx wrap staging"):
                    for j in range(2):
                        nc.scalar.dma_start(
                            stg[0:32, 32 * j : 32 * j + 16],
                            idx16[r : r + 1, j * 512 : (j + 1) * 512].rearrange(
                                "p (a b) -> p a b", b=16
                            ),
                        )
                nc.vector.tensor_copy(stg_j[:, :, 16:32], stg_j[:, :, 0:16])
                idxw = spool.tile([128, 64], I16, tag="idxw")
                for g in range(4):
                    nc.vector.transpose(idxw[32 * g : 32 * (g + 1), :], stg[:])
                idxw_r.append(idxw)

            # gathers split in batch halves, interleaved r0/r1, so blocks
            # c=0-3 of BOTH tensors arrive after the first two half-gathers.
            # g2[r][hf][p, c, :] = mem[idx_{(4*hf+c)*128+p}, :]  (bf16 rows)
            HB = BL // 2
            g2_tiles = [[None, None], [None, None]]
            for hf in range(2):
                for r in range(2):
                    g2 = gpool.tile([128, HB // 128, H], BF16, tag=f"g2_{r}{hf}")
                    nc.gpsimd.dma_gather(
                        out_ap=g2[:],
                        in_ap=mem_t[:],
                        idxs_ap=idxw_r[r][:, hf * 32 : (hf + 1) * 32],
                        num_idxs=HB,
                        num_idxs_reg=HB,
                        elem_size=H,
                        transpose=False,
                    )
                    g2_tiles[r][hf] = g2

            # ---------------- main matmuls + epilogue ----------------
            srcs_xh = [(x8_sb, 0), (h8_sb, 1)]
            ps_tiles = {}
            mem_sb = [[None] * MT, [None] * MT]

            def emit_transpose(c):
                # PE-transpose gathered rows of batch-block c into
                # [feat, batch] bf16 tiles; PSUM->SBUF copies alternate
                # DVE / ACT so neither engine rate-limits the pipeline
                for r in range(2):
                    g2 = g2_tiles[r][c // 4]
                    cc = c % 4
                    mt = gpool.tile([128, KC, 128], BF16, tag=f"mem{r}_{c}")
                    for k in range(KC):
                        tp = pp.tile([128, 128], BF16, tag="sm", bufs=2)
                        nc.tensor.transpose(
                            tp[:], g2[:, cc, k * 128 : (k + 1) * 128],
                            ident_sb[:],
                        )
                        if k % 2 == 0:
                            nc.vector.tensor_copy(mt[:, k, :], tp[:])
                        else:
                            nc.scalar.activation(
                                mt[:, k, :], tp[:],
                                mybir.ActivationFunctionType.Identity,
                            )
                    mem_sb[r][c] = mt

            def emit_xh(m):
                ps = pp.tile([128, H], F32, tag="acc", bufs=3)
                ps_tiles[m] = ps
                ms = slice(m * 128, (m + 1) * 128)
                # rank-1 bias matmul opens each bank's accumulation group
                for n in range(H // 512):
                    nc.tensor.matmul(
                        ps[:, n * 512 : (n + 1) * 512],
                        ones_sb[:],
                        bias16_sb[:, n * 512 : (n + 1) * 512],
                        start=True,
                        stop=False,
                    )
                for si, (act, wi) in enumerate(srcs_xh):
                    for k in range(KC):
                        lhs = act[:, k, ms]
                        for n in range(H // 512):
                            nc.tensor.matmul(
                                ps[:, n * 512 : (n + 1) * 512],
                                lhs,
                                w_sb[wi][:, k, n * 512 : (n + 1) * 512],
                                start=False,
                                stop=False,
                            )

            def emit_mem_epilogue(m):
                ps = ps_tiles.pop(m)
                ms = slice(m * 128, (m + 1) * 128)
                st6 = epool.tile([128, 2, 6], F32, tag="st6")
                # bank-outer: bank 0 stops (and its bn_stats runs) while
                # bank 1's matmuls are still streaming
                for n in range(H // 512):
                    for si in range(2):
                        mt = mem_sb[si][m]  # [128, KC, 128] block for this m
                        for k in range(KC):
                            nc.tensor.matmul(
                                ps[:, n * 512 : (n + 1) * 512],
                                mt[:, k, :],
                                w_sb[2 + si][:, k, n * 512 : (n + 1) * 512],
                                start=False,
                                stop=(si == 1 and k == KC - 1),
                            )
                    nc.vector.bn_stats(
                        st6[:, n, :], ps[:, n * 512 : (n + 1) * 512]
                    )
                mv = epool.tile([128, 2], F32, tag="mv")
                nc.vector.bn_aggr(mv[:], st6.rearrange("p a b -> p (a b)"))
                # rstd = 1/sqrt(var+eps) entirely on DVE:
                # quake seed from vh=(var+eps)/2 bits, then 2 Newton steps
                # y <- y*(1.5 - vh*y^2).
                st = epool.tile([128, 4], F32, tag="rs")
                vh = st[:, 0:1]
                y = st[:, 1:2]
                a_ = st[:, 2:3]
                nmu = st[:, 3:4]
                nc.vector.tensor_scalar(
                    vh, mv[:, 1:2], 0.5, EPS_SC * 0.5,
                    mybir.AluOpType.mult, mybir.AluOpType.add,
                )
                nc.vector.tensor_scalar(
                    a_.bitcast(I32), vh.bitcast(I32), 1, None,
                    mybir.AluOpType.logical_shift_right,
                )
                nc.vector.tensor_tensor(
                    y.bitcast(I32), magic_sb[:], a_.bitcast(I32),
                    mybir.AluOpType.subtract,
                )
                for _ in range(2):
                    nc.vector.tensor_tensor(a_, y, y, mybir.AluOpType.mult)
                    nc.vector.tensor_tensor(a_, a_, vh, mybir.AluOpType.mult)
                    nc.vector.tensor_scalar(
                        a_, a_, 1.5, -1.0,
                        mybir.AluOpType.subtract, mybir.AluOpType.mult,
                    )
                    nc.vector.tensor_tensor(y, y, a_, mybir.AluOpType.mult)
                nc.vector.tensor_scalar(
                    nmu, mv[:, 0:1], y, -1.0,
                    mybir.AluOpType.mult, mybir.AluOpType.mult,
                )
                o = epool.tile([128, H], BF16, tag="o")
                if trivial_gb:
                    # out = sigmoid((t - mu) * rstd), read from PSUM
                    nc.scalar.activation(
                        o[:], ps[:], mybir.ActivationFunctionType.Sigmoid,
                        bias=nmu, scale=y,
                    )
                else:
                    xh = epool.tile([128, H], F32, tag="xh")
                    nc.scalar.activation(
                        xh[:], ps[:], mybir.ActivationFunctionType.Identity,
                        bias=nmu, scale=y,
                    )
                    nc.vector.tensor_tensor(
                        xh[:], xh[:], gam_sb[:], mybir.AluOpType.mult
                    )
                    nc.vector.tensor_tensor(
                        xh[:], xh[:], bet_sb[:], mybir.AluOpType.add
                    )
                    nc.scalar.activation(
                        o[:], xh[:], mybir.ActivationFunctionType.Sigmoid,
                        bias=zero_sb[:, 0:1],
                    )
                nc.sync.dma_start(out_t[ms, :], o[:])

            # "acc" slot ring: L(s0) x0(s1) x1(s2) x2(s0) x3(s1) x4(s2)
            # x5(s0) x6(s1) x7(s2); each reuse is covered by >=1 emitted
            # mem-tile of PE work so the epilogue chain never stalls the PE.
            emit_xh(0)
            emit_xh(1)
            emit_xh(2)
            for c in range(4):
                emit_transpose(c)
            emit_mem_epilogue(0)
            emit_mem_epilogue(1)
            emit_xh(3)
            emit_xh(4)
            for c in range(4, 8):
                emit_transpose(c)
            emit_mem_epilogue(2)
            emit_mem_epilogue(3)
            emit_xh(5)
            emit_xh(6)
            emit_mem_epilogue(4)
            emit_mem_epilogue(5)
            emit_xh(7)
            emit_mem_epilogue(6)
            emit_mem_epilogue(7)

    nc.compile()  # bacc register allocation / DCE
    return nc


def _to_kxp(a, dtype):
    """[batch, feat] -> [128, KC, batch] with feat = k*128 + p."""
    t = np.ascontiguousarray(a.T.reshape(KC, 128, -1).transpose(1, 0, 2))
    return t.astype(dtype)


def prep(inputs):
    """Host-side shard/layout prep. Returns (in_maps, trivial_gb)."""
    x = np.asarray(inputs["x"], np.float32)
    h = np.asarray(inputs["h_prev"], np.float32)
    memory = np.asarray(inputs["memory"], np.float32)
    gamma = np.asarray(inputs["gamma"], np.float32)
    beta = np.asarray(inputs["beta"], np.float32)
    trivial_gb = bool(np.all(gamma == 1.0) and np.all(beta == 0.0))

    bf = ml_dtypes.bfloat16
    e4 = ml_dtypes.float8_e4m3
    # W is [out, in]; the kernel wants w[p, k, n] = W[n, k*128+p], which is
    # exactly _to_kxp applied to W with (out, in) in the (batch, feat) slots.
    w_cat = np.stack(
        [
            _to_kxp(np.asarray(inputs[n], np.float32) * WSCALE, bf)
            for n in ("Ww", "Uw", "Qrw", "Qlw")
        ]
    )
    mw = _to_kxp(np.asarray(inputs["Mw"], np.float32), np.float16)

    pw = np.zeros((2 * NB, 2), np.float32)
    pw[:NB, 0] = 2.0 ** np.arange(NB - 1, -1, -1)
    pw[NB:, 1] = 2.0 ** np.arange(NB - 1, -1, -1)
    clip = np.array(
        [[0.0, MEM // 2 - 1], [MEM // 2, MEM - 1]], np.float32
    )  # [row, (lo, hi)]

    mem16 = memory.astype(bf)
    bias16 = (
        (
            np.asarray(inputs["Wb"], np.float32)
            + np.asarray(inputs["Ub"], np.float32)
            + np.asarray(inputs["Qrb"], np.float32)
            + np.asarray(inputs["Qlb"], np.float32)
        )
        * WSCALE
    ).astype(bf).reshape(1, H)

    # pack the small constants into one [128, NCONST] f32 buffer
    const = np.zeros((128, NCONST), np.float32)
    const[:2, C_CLIP : C_CLIP + 2] = clip
    const[: 2 * NB, C_NEGMB : C_NEGMB + 1] = -np.asarray(
        inputs["Mb"], np.float32
    ).reshape(2 * NB, 1)
    const[: 2 * NB, C_PW : C_PW + 1] = pw.astype(bf).view(np.float32)
    const[:, C_MAGIC : C_MAGIC + 1] = (
        np.full((128, 1), RSQRT_MAGIC, np.int32).view(np.float32)
    )
    ident16 = np.eye(128, dtype=np.float32).astype(bf)
    const[:, C_IDENT : C_IDENT + 64] = ident16.view(np.float32)

    common = dict(
        w16_t=w_cat, bias16_t=bias16, const_t=const, mem_t=mem16, mw_t=mw
    )
    if not trivial_gb:
        common["gam_t"] = np.ascontiguousarray(np.broadcast_to(gamma, (128, H)))
        common["bet_t"] = np.ascontiguousarray(np.broadcast_to(beta, (128, H)))

    in_maps = []
    for c in range(NCORES):
        xs = x[c * BL : (c + 1) * BL]
        hs = h[c * BL : (c + 1) * BL]
        in_maps.append(
            dict(
                x8_t=_to_kxp(xs, e4),
                h8_t=_to_kxp(hs, e4),
                h16_t=_to_kxp(hs, np.float16),
                **common,
            )
        )
    return in_maps, trivial_gb


def get_nc(trivial_gb):
    key = ("nc", trivial_gb)
    if key not in _CACHE:
        _CACHE[key] = _build(trivial_gb)
    return _CACHE[key]


def run(inputs, trace=False, **kw):
    in_maps, trivial_gb = prep(inputs)
    nc = get_nc(trivial_gb)
    res = run_bass_kernel_spmd(
        nc, in_maps, core_ids=list(range(NCORES)), trace=trace, **kw
    )
    out = np.concatenate([res.results[c]["out_t"] for c in range(NCORES)], axis=0)
    return out.astype(np.float32), res


def kernel(**inputs):
    return run(inputs)[0]


# revision 48
# speedup vs baseline: 1.1818x; 1.0038x over previous
"""Trainium2 Bass kernel for nn_BinaryMemoryRNN (scatter_memory).

Computation (reference):
    logits = h_prev @ Mw.T + Mb                 # [B, 28]
    b1/b2  = bits of logits halves (> 0)
    idx1   = clip(sum(b1 * 2^(13-j)), 0, 8191)
    idx2   = clip(sum(b2 * 2^(13-j)), 8192, 16383)
    pre    = x @ Ww.T + h_prev @ Uw.T + mem[idx1] @ Qrw.T + mem[idx2] @ Qlw.T + bias
    out    = sigmoid(layernorm(pre) * gamma + beta)

Strategy: data-parallel over batch across 8 cores (1024 rows each).
  - x/h activations in fp8-e4m3 as the stationary matmul operand; weights
    stream as bf16 (the fp8xfp8 combination runs at half rate on TRN2, and
    fp8 as the moving operand is slow - fp8-stationary x bf16-moving is
    full rate). Weights scaled x256 so fp8 activations meet them in a
    range the layernorm renormalizes away; only the bias needs the x256.
  - logits matmul in fp16 (sign-sensitive index bits: fp16's 10 mantissa
    bits flip ~1e-3 of them; full fp32 would cost 4x the PE time and 2x
    the critical-path DMA).
  - memory table replicated in DRAM as bf16 [16384, 1024]; rows fetched
    with gpsimd.dma_gather (row layout), PE-transposed to [feat, batch].
    PSUM->SBUF copies of transposed tiles alternate DVE / ACT engines.
  - bias enters the PSUM accumulation as a rank-1 (ones x bias) matmul,
    so the epilogue reads layernorm stats straight from PSUM; mem matmuls
    are emitted bank-outer so bank-0 stats overlap bank-1 matmuls.
  - rstd = 1/sqrt(var+eps) via DVE quake-seed + 2 Newton steps: the ACT
    engine's activation table never leaves Sigmoid. Output written bf16.
  - DMA issue order doubles as the priority schedule: logits path first
    (mw, h16 pieces), then x/w0 for the first matmuls, then the rest.
"""

import sys

sys.path.insert(0, "/opt/trn_rl_repo")

from contextlib import ExitStack

import numpy as np
import ml_dtypes

import concourse.bass as bass
import concourse.tile as tile
from concourse import bacc, mybir, library_config
from concourse.bass_utils import run_bass_kernel_spmd

F32 = mybir.dt.float32
F16 = mybir.dt.float16
BF16 = mybir.dt.bfloat16
F8E4 = mybir.dt.float8e4
I16 = mybir.dt.int16
I32 = mybir.dt.int32

B, I, H, NB = 8192, 1024, 1024, 14
MEM = 2**NB
NCORES = 8
BL = B // NCORES  # 1024 batch rows per core
KC = H // 128  # 8 contraction chunks
MT = BL // 128  # 8 output row-tiles per core
EPS = 1e-5
WSCALE = 256.0
EPS_SC = EPS * WSCALE * WSCALE  # eps for the x256-scaled pre-activation
RSQRT_MAGIC = 0x5EF759DF  # 0x5f3759df - 0x00400000: seed for rsqrt(2*vh)

# const_t packed layout (f32 columns)
C_CLIP = 0  # [2, 2] idx clip bounds
C_NEGMB = 2  # [28, 1] -Mb
C_PW = 3  # [28, 1] powers of two as bf16 pair-packed in f32
C_MAGIC = 4  # [128, 1] rsqrt seed magic (int32 bits)
C_IDENT = 5  # [128, 64] 128x128 bf16 identity (bitcast)
NCONST = 69

_CACHE = {}


def _build(trivial_gb: bool):
    """Trace the Bass/Tile module (shared by all 8 cores, SPMD)."""
    nc = bacc.Bacc(
        "TRN2", target_bir_lowering=False, debug=False, enable_asserts=True
    )

    x8_t = nc.dram_tensor("x8_t", [128, KC, BL], F8E4, kind="ExternalInput").ap()
    h8_t = nc.dram_tensor("h8_t", [128, KC, BL], F8E4, kind="ExternalInput").ap()
    h16_t = nc.dram_tensor("h16_t", [128, KC, BL], F16, kind="ExternalInput").ap()
    mw_t = nc.dram_tensor("mw_t", [128, KC, 2 * NB], F16, kind="ExternalInput").ap()
    # weights, [src, feat_in(part), feat_in(chunk), feat_out]; src order W,U,Qr,Ql
    w16_t = nc.dram_tensor("w16_t", [4, 128, KC, H], BF16, kind="ExternalInput").ap()
    bias16_t = nc.dram_tensor("bias16_t", [1, H], BF16, kind="ExternalInput").ap()
    const_t = nc.dram_tensor("const_t", [128, NCONST], F32, kind="ExternalInput").ap()
    mem_t = nc.dram_tensor("mem_t", [MEM, H], BF16, kind="ExternalInput").ap()
    if not trivial_gb:
        gam_t = nc.dram_tensor("gam_t", [128, H], F32, kind="ExternalInput").ap()
        bet_t = nc.dram_tensor("bet_t", [128, H], F32, kind="ExternalInput").ap()
    out_t = nc.dram_tensor("out_t", [BL, H], BF16, kind="ExternalOutput").ap()

    with tile.TileContext(nc) as tc:
        with ExitStack() as ctx:
            # ---------------- pools ----------------
            cpool = ctx.enter_context(tc.tile_pool(name="consts", bufs=1))
            apool = ctx.enter_context(tc.tile_pool(name="acts", bufs=1))
            hpool = ctx.enter_context(tc.tile_pool(name="h16", bufs=4))
            gpool = ctx.enter_context(tc.tile_pool(name="gathered", bufs=1))
            spool = ctx.enter_context(tc.tile_pool(name="small", bufs=2))
            epool = ctx.enter_context(tc.tile_pool(name="epilogue", bufs=2))
            # PSUM: tag "acc" rotates 3 two-bank slots (logits shares the
            # ring); tag "sm" rotates 2 one-bank slots (idx halves + PE
            # transposes). 3*2 + 2*1 = 8 banks.
            pp = ctx.enter_context(tc.tile_pool(name="psum", bufs=1, space="PSUM"))

            # gpsimd ucode library containing DMAGatherAnt; load it up front
            # so the Q7 IRAM reload overlaps the initial DMAs.
            nc.gpsimd.load_library(library_config.attnmlp)

            # ---------------- input loads (issue order = priority) ----------
            mw_sb = cpool.tile([128, KC, 2 * NB], F16, tag="mw")
            nc.sync.dma_start(mw_sb[:], mw_t[:])
            const_sb = cpool.tile([128, NCONST], F32, tag="const")
            nc.sync.dma_start(const_sb[:], const_t[:])
            clip_sb = const_sb[0:2, C_CLIP : C_CLIP + 2]
            negmb_sb = const_sb[0 : 2 * NB, C_NEGMB : C_NEGMB + 1]
            pw_sb = const_sb[0 : 2 * NB, C_PW : C_PW + 1].bitcast(BF16)
            magic_sb = const_sb[:, C_MAGIC : C_MAGIC + 1].bitcast(I32)
            ident_sb = const_sb[:, C_IDENT : C_IDENT + 64].bitcast(BF16)

            # warm the Sigmoid activation table while DMAs run (the only
            # ACT table in this kernel -> one table load total)
            warm = cpool.tile([128, 1], F32, tag="warm")
            nc.vector.memset(warm[:], 0.0)
            nc.scalar.activation(
                warm[:], warm[:], mybir.ActivationFunctionType.Sigmoid
            )
            ones_sb = cpool.tile([1, 128], BF16, tag="ones")
            nc.vector.memset(ones_sb[:], 1.0)

            # h16 for logits in 4 pieces, interleaved with x8/w0 so the
            # first xh matmuls aren't starved behind the full logits load
            h16p = []
            for piece in range(4):
                hp = hpool.tile([128, 2, BL], F16, tag="slab")
                h16p.append(hp)
            x8_sb = apool.tile([128, KC, BL], F8E4, tag="x8")
            h8_sb = apool.tile([128, KC, BL], F8E4, tag="h8")
            w_sb = [
                cpool.tile([128, KC, H], BF16, tag=f"w{s}", name=f"w{s}")
                for s in range(4)
            ]
            bias16_sb = cpool.tile([1, H], BF16, tag="bias16")

            for piece in range(4):
                nc.sync.dma_start(
                    h16p[piece][:], h16_t[:, 2 * piece : 2 * piece + 2, :]
                )
            nc.sync.dma_start(bias16_sb[:], bias16_t[:])
            # x and weights follow in k-chunk pieces so the xh matmuls can
            # start on partial data right after logits
            nc.sync.dma_start(x8_sb[:, 0:4, :], x8_t[:, 0:4, :])
            nc.sync.dma_start(x8_sb[:, 4:8, :], x8_t[:, 4:8, :])
            for kk in range(4):
                nc.sync.dma_start(
                    w_sb[0][:, 2 * kk : 2 * kk + 2, :],
                    w16_t[0, :, 2 * kk : 2 * kk + 2, :],
                )
            nc.sync.dma_start(h8_sb[:, 0:4, :], h8_t[:, 0:4, :])
            nc.sync.dma_start(h8_sb[:, 4:8, :], h8_t[:, 4:8, :])
            for kk in range(4):
                nc.sync.dma_start(
                    w_sb[1][:, 2 * kk : 2 * kk + 2, :],
                    w16_t[1, :, 2 * kk : 2 * kk + 2, :],
                )
            for s in (2, 3):
                for kk in range(4):
                    nc.sync.dma_start(
                        w_sb[s][:, 2 * kk : 2 * kk + 2, :],
                        w16_t[s, :, 2 * kk : 2 * kk + 2, :],
                    )
            if not trivial_gb:
                gam_sb = cpool.tile([128, H], F32, tag="gam")
                nc.sync.dma_start(gam_sb[:], gam_t[:])
                bet_sb = cpool.tile([128, H], F32, tag="bet")
                nc.sync.dma_start(bet_sb[:], bet_t[:])
                zero_sb = cpool.tile([128, 1], F32, tag="zero")
                nc.vector.memset(zero_sb[:], 0.0)

            def h16_chunk(k):
                return h16p[k // 2][:, k % 2, :]

            # ---------------- index pipeline ----------------
            # logits.T [28, BL] fp16 inputs, fp32 PSUM. Bank-outer (batch
            # halves): bank 0's bits/idx/wrap/gathers launch while bank 1's
            # logits matmuls are still streaming.
            logit_ps = pp.tile([2 * NB, BL], F32, tag="acc", bufs=3)
            bits_sb = spool.tile([2 * NB, BL], BF16, tag="bits")
            idx16 = spool.tile([2, BL], I16, tag="idx16")
            stg_r = []
            idxw_r = []
            for r in range(2):
                stg = spool.tile([32, 64], I16, tag="stage", name=f"stg{r}")
                stg_r.append(stg)
                idxw = spool.tile([128, 64], I16, tag="idxw", name=f"idxw{r}")
                idxw_r.append(idxw)
            HB = BL // 2
            g2_tiles = [[None, None], [None, None]]

            for n in range(2):
                sl = slice(n * 512, (n + 1) * 512)
                for k in range(KC):
                    nc.tensor.matmul(
                        logit_ps[:, sl],
                        mw_sb[:, k, :],
                        h16_chunk(k)[:, sl],
                        start=(k == 0),
                        stop=(k == KC - 1),
                    )
                # bits = (h@Mw.T + Mb > 0)  <=>  (h@Mw.T > -Mb), as 1.0/0.0
                nc.vector.tensor_scalar(
                    bits_sb[:, sl], logit_ps[:, sl], negmb_sb[:, 0:1], None,
                    mybir.AluOpType.is_gt,
                )
                # raw indices via tiny matmul with powers of two: [2, 512]
                idx_ps = pp.tile([2, 512], F32, tag="sm", bufs=2)
                nc.tensor.matmul(
                    idx_ps[:], pw_sb, bits_sb[:, sl], start=True, stop=True
                )
                # clip + cast to int16; per-partition clip bounds:
                # row0 -> [0, 8191], row1 -> [8192, 16383]
                nc.vector.tensor_scalar(
                    idx16[:, sl], idx_ps[:],
                    clip_sb[:, 0:1], clip_sb[:, 1:2],
                    mybir.AluOpType.max, mybir.AluOpType.min,
                )
                # wrap this batch-half into the [16-group, 32] gather layout:
                # stage S[i, 32n+q'] = idx[(32n+i)*16+q'%16] via a strided
                # DMA (16 cols, duplicated), then 32x32 DVE transposes into
                # idxw columns [32n:32n+32]
                for r in range(2):
                    stg = stg_r[r]
                    stg_j = stg[0:32, :].rearrange("p (j hq) -> p j hq", j=2)
                    with nc.allow_non_contiguous_dma(
                        reason="tiny idx wrap staging"
                    ):
                        nc.scalar.dma_start(
                            stg[0:32, 32 * n : 32 * n + 16],
                            idx16[r : r + 1, sl].rearrange(
                                "p (a b) -> p a b", b=16
                            ),
                        )
                    nc.vector.tensor_copy(
                        stg_j[:, n, 16:32], stg_j[:, n, 0:16]
                    )
                    for g in range(4):
                        nc.vector.transpose(
                            idxw_r[r][32 * g : 32 * (g + 1),
                                      32 * n : 32 * n + 32],
                            stg[:, 32 * n : 32 * n + 32],
                        )
                # launch this half's gathers immediately:
                # g2[r][n][p, c, :] = mem[idx_{(4*n+c)*128+p}, :] (bf16 rows)
                for r in range(2):
                    g2 = gpool.tile(
                        [128, HB // 128, H], BF16, tag=f"g2_{r}{n}",
                        name=f"g2_{r}{n}",
                    )
                    nc.gpsimd.dma_gather(
                        out_ap=g2[:],
                        in_ap=mem_t[:],
                        idxs_ap=idxw_r[r][:, n * 32 : (n + 1) * 32],
                        num_idxs=HB,
                        num_idxs_reg=HB,
                        elem_size=H,
                        transpose=False,
                    )
                    g2_tiles[r][n] = g2

            # ---------------- main matmuls + epilogue ----------------
            srcs_xh = [(x8_sb, 0), (h8_sb, 1)]
            ps_tiles = {}
            mem_sb = [[None] * MT, [None] * MT]

            def emit_transpose(c):
                # PE-transpose gathered rows of batch-block c into
                # [feat, batch] bf16 tiles; PSUM->SBUF copies alternate
                # DVE / ACT so neither engine rate-limits the pipeline
                for r in range(2):
                    g2 = g2_tiles[r][c // 4]
                    cc = c % 4
                    mt = gpool.tile([128, KC, 128], BF16, tag=f"mem{r}_{c}")
                    for k in range(KC):
                        tp = pp.tile([128, 128], BF16, tag="sm", bufs=2)
                        nc.tensor.transpose(
                            tp[:], g2[:, cc, k * 128 : (k + 1) * 128],
                            ident_sb[:],
                        )
                        if k % 2 == 0:
                            nc.vector.tensor_copy(mt[:, k, :], tp[:])
                        else:
                            nc.scalar.activation(
                                mt[:, k, :], tp[:],
                                mybir.ActivationFunctionType.Identity,
                            )
                    mem_sb[r][c] = mt

            def emit_xh(m):
                ps = pp.tile([128, H], F32, tag="acc", bufs=3)
                ps_tiles[m] = ps
                ms = slice(m * 128, (m + 1) * 128)
                # rank-1 bias matmul opens each bank's accumulation group
                for n in range(H // 512):
                    nc.tensor.matmul(
                        ps[:, n * 512 : (n + 1) * 512],
                        ones_sb[:],
                        bias16_sb[:, n * 512 : (n + 1) * 512],
                        start=True,
                        stop=False,
                    )
                for si, (act, wi) in enumerate(srcs_xh):
                    for k in range(KC):
                        lhs = act[:, k, ms]
                        for n in range(H // 512):
                            nc.tensor.matmul(
                                ps[:, n * 512 : (n + 1) * 512],
                                lhs,
                                w_sb[wi][:, k, n * 512 : (n + 1) * 512],
                                start=False,
                                stop=False,
                            )

            def emit_mem_epilogue(m):
                ps = ps_tiles.pop(m)
                ms = slice(m * 128, (m + 1) * 128)
                st6 = epool.tile([128, 2, 6], F32, tag="st6")
                # bank-outer: bank 0 stops (and its bn_stats runs) while
                # bank 1's matmuls are still streaming
                for n in range(H // 512):
                    for si in range(2):
                        mt = mem_sb[si][m]  # [128, KC, 128] block for this m
                        for k in range(KC):
                            nc.tensor.matmul(
                                ps[:, n * 512 : (n + 1) * 512],
                                mt[:, k, :],
                                w_sb[2 + si][:, k, n * 512 : (n + 1) * 512],
                                start=False,
                                stop=(si == 1 and k == KC - 1),
                            )
                    nc.vector.bn_stats(
                        st6[:, n, :], ps[:, n * 512 : (n + 1) * 512]
                    )
                mv = epool.tile([128, 2], F32, tag="mv")
                nc.vector.bn_aggr(mv[:], st6.rearrange("p a b -> p (a b)"))
                # rstd = 1/sqrt(var+eps) entirely on DVE:
                # quake seed from vh=(var+eps)/2 bits, then 2 Newton steps
                # y <- y*(1.5 - vh*y^2).
                st = epool.tile([128, 4], F32, tag="rs")
                vh = st[:, 0:1]
                y = st[:, 1:2]
                a_ = st[:, 2:3]
                nmu = st[:, 3:4]
                nc.vector.tensor_scalar(
                    vh, mv[:, 1:2], 0.5, EPS_SC * 0.5,
                    mybir.AluOpType.mult, mybir.AluOpType.add,
                )
                nc.vector.tensor_scalar(
                    a_.bitcast(I32), vh.bitcast(I32), 1, None,
                    mybir.AluOpType.logical_shift_right,
                )
                nc.vector.tensor_tensor(
                    y.bitcast(I32), magic_sb[:], a_.bitcast(I32),
                    mybir.AluOpType.subtract,
                )
                for _ in range(2):
                    nc.vector.tensor_tensor(a_, y, y, mybir.AluOpType.mult)
                    nc.vector.tensor_tensor(a_, a_, vh, mybir.AluOpType.mult)
                    nc.vector.tensor_scalar(
                        a_, a_, 1.5, -1.0,
                        mybir.AluOpType.subtract, mybir.AluOpType.mult,
                    )
                    nc.vector.tensor_tensor(y, y, a_, mybir.AluOpType.mult)
                nc.vector.tensor_scalar(
                    nmu, mv[:, 0:1], y, -1.0,
                    mybir.AluOpType.mult, mybir.AluOpType.mult,
                )
                o = epool.tile([128, H], BF16, tag="o")
                if trivial_gb:
                    # out = sigmoid((t - mu) * rstd), read from PSUM
                    nc.scalar.activation(
                        o[:], ps[:], mybir.ActivationFunctionType.Sigmoid,
                        bias=nmu, scale=y,
                    )
                else:
                    xh = epool.tile([128, H], F32, tag="xh")
                    nc.scalar.activation(
                        xh[:], ps[:], mybir.ActivationFunctionType.Identity,
                        bias=nmu, scale=y,
                    )
                    nc.vector.tensor_tensor(
                        xh[:], xh[:], gam_sb[:], mybir.AluOpType.mult
                    )
                    nc.vector.tensor_tensor(
                        xh[:], xh[:], bet_sb[:], mybir.AluOpType.add
                    )
                    nc.scalar.activation(
                        o[:], xh[:], mybir.ActivationFunctionType.Sigmoid,
                        bias=zero_sb[:, 0:1],
                    )
                nc.sync.dma_start(out_t[ms, :], o[:])

            # "acc" slot ring: L(s0) x0(s1) x1(s2) x2(s0) x3(s1) x4(s2)
            # x5(s0) x6(s1) x7(s2); each reuse is covered by >=1 emitted
            # mem-tile of PE work so the epilogue chain never stalls the PE.
            emit_xh(0)
            emit_xh(1)
            emit_xh(2)
            for c in range(4):
                emit_transpose(c)
            emit_mem_epilogue(0)
            emit_mem_epilogue(1)
            emit_xh(3)
            emit_xh(4)
            for c in range(4, 8):
                emit_transpose(c)
            emit_mem_epilogue(2)
            emit_mem_epilogue(3)
            emit_xh(5)
            emit_xh(6)
            emit_mem_epilogue(4)
            emit_mem_epilogue(5)
            emit_xh(7)
            emit_mem_epilogue(6)
            emit_mem_epilogue(7)

    nc.compile()  # bacc register allocation / DCE
    return nc


def _to_kxp(a, dtype):
    """[batch, feat] -> [128, KC, batch] with feat = k*128 + p."""
    t = np.ascontiguousarray(a.T.reshape(KC, 128, -1).transpose(1, 0, 2))
    return t.astype(dtype)


def prep(inputs):
    """Host-side shard/layout prep. Returns (in_maps, trivial_gb)."""
    x = np.asarray(inputs["x"], np.float32)
    h = np.asarray(inputs["h_prev"], np.float32)
    memory = np.asarray(inputs["memory"], np.float32)
    gamma = np.asarray(inputs["gamma"], np.float32)
    beta = np.asarray(inputs["beta"], np.float32)
    trivial_gb = bool(np.all(gamma == 1.0) and np.all(beta == 0.0))

    bf = ml_dtypes.bfloat16
    e4 = ml_dtypes.float8_e4m3
    # W is [out, in]; the kernel wants w[p, k, n] = W[n, k*128+p], which is
    # exactly _to_kxp applied to W with (out, in) in the (batch, feat) slots.
    w_cat = np.stack(
        [
            _to_kxp(np.asarray(inputs[n], np.float32) * WSCALE, bf)
            for n in ("Ww", "Uw", "Qrw", "Qlw")
        ]
    )
    mw = _to_kxp(np.asarray(inputs["Mw"], np.float32), np.float16)

    pw = np.zeros((2 * NB, 2), np.float32)
    pw[:NB, 0] = 2.0 ** np.arange(NB - 1, -1, -1)
    pw[NB:, 1] = 2.0 ** np.arange(NB - 1, -1, -1)
    clip = np.array(
        [[0.0, MEM // 2 - 1], [MEM // 2, MEM - 1]], np.float32
    )  # [row, (lo, hi)]

    mem16 = memory.astype(bf)
    bias16 = (
        (
            np.asarray(inputs["Wb"], np.float32)
            + np.asarray(inputs["Ub"], np.float32)
            + np.asarray(inputs["Qrb"], np.float32)
            + np.asarray(inputs["Qlb"], np.float32)
        )
        * WSCALE
    ).astype(bf).reshape(1, H)

    # pack the small constants into one [128, NCONST] f32 buffer
    const = np.zeros((128, NCONST), np.float32)
    const[:2, C_CLIP : C_CLIP + 2] = clip
    const[: 2 * NB, C_NEGMB : C_NEGMB + 1] = -np.asarray(
        inputs["Mb"], np.float32
    ).reshape(2 * NB, 1)
    const[: 2 * NB, C_PW : C_PW + 1] = pw.astype(bf).view(np.float32)
    const[:, C_MAGIC : C_MAGIC + 1] = (
        np.full((128, 1), RSQRT_MAGIC, np.int32).view(np.float32)
    )
    ident16 = np.eye(128, dtype=np.float32).astype(bf)
    const[:, C_IDENT : C_IDENT + 64] = ident16.view(np.float32)

    common = dict(
        w16_t=w_cat, bias16_t=bias16, const_t=const, mem_t=mem16, mw_t=mw
    )
    if not trivial_gb:
        common["gam_t"] = np.ascontiguousarray(np.broadcast_to(gamma, (128, H)))
        common["bet_t"] = np.ascontiguousarray(np.broadcast_to(beta, (128, H)))

    in_maps = []
    for c in range(NCORES):
        xs = x[c * BL : (c + 1) * BL]
        hs = h[c * BL : (c + 1) * BL]
        in_maps.append(
            dict(
                x8_t=_to_kxp(xs, e4),
                h8_t=_to_kxp(hs, e4),
                h16_t=_to_kxp(hs, np.float16),
                **common,
            )
        )
    return in_maps, trivial_gb


def get_nc(trivial_gb):
    key = ("nc", trivial_gb)
    if key not in _CACHE:
        _CACHE[key] = _build(trivial_gb)
    return _CACHE[key]


def run(inputs, trace=False, **kw):
    in_maps, trivial_gb = prep(inputs)
    nc = get_nc(trivial_gb)
    res = run_bass_kernel_spmd(
        nc, in_maps, core_ids=list(range(NCORES)), trace=trace, **kw
    )
    out = np.concatenate([res.results[c]["out_t"] for c in range(NCORES)], axis=0)
    return out.astype(np.float32), res


def kernel(**inputs):
    return run(inputs)[0]


# revision 49
# speedup vs baseline: 1.3447x; 1.1378x over previous
"""Trainium2 Bass kernel for nn_BinaryMemoryRNN (scatter_memory).

Computation (reference):
    logits = h_prev @ Mw.T + Mb                 # [B, 28]
    b1/b2  = bits of logits halves (> 0)
    idx1   = clip(sum(b1 * 2^(13-j)), 0, 8191)
    idx2   = clip(sum(b2 * 2^(13-j)), 8192, 16383)
    pre    = x @ Ww.T + h_prev @ Uw.T + mem[idx1] @ Qrw.T + mem[idx2] @ Qlw.T + bias
    out    = sigmoid(layernorm(pre) * gamma + beta)

Strategy: data-parallel over batch across 8 cores (1024 rows each).
  - x/h activations in fp8-e4m3 as the stationary matmul operand; weights
    stream as bf16 (the fp8xfp8 combination runs at half rate on TRN2, and
    fp8 as the moving operand is slow - fp8-stationary x bf16-moving is
    full rate). Weights scaled x256 so fp8 activations meet them in a
    range the layernorm renormalizes away; only the bias needs the x256.
  - logits matmul in fp16 (sign-sensitive index bits: fp16's 10 mantissa
    bits flip ~1e-3 of them; full fp32 would cost 4x the PE time and 2x
    the critical-path DMA).
  - memory table replicated in DRAM as bf16 [16384, 1024]; rows fetched
    with gpsimd.dma_gather (row layout), PE-transposed to [feat, batch].
    PSUM->SBUF copies of transposed tiles alternate DVE / ACT engines.
  - bias enters the PSUM accumulation as a rank-1 (ones x bias) matmul,
    so the epilogue reads layernorm stats straight from PSUM; mem matmuls
    are emitted bank-outer so bank-0 stats overlap bank-1 matmuls.
  - rstd = 1/sqrt(var+eps) via DVE quake-seed + 2 Newton steps: the ACT
    engine's activation table never leaves Sigmoid. Output written bf16.
  - DMA issue order doubles as the priority schedule: logits path first
    (mw, h16 pieces), then x/w0 for the first matmuls, then the rest.
"""

import sys

sys.path.insert(0, "/opt/trn_rl_repo")

from contextlib import ExitStack

import numpy as np
import ml_dtypes

import concourse.bass as bass
import concourse.tile as tile
from concourse import bacc, mybir, library_config
from concourse.bass_utils import run_bass_kernel_spmd

F32 = mybir.dt.float32
F16 = mybir.dt.float16
BF16 = mybir.dt.bfloat16
F8E4 = mybir.dt.float8e4
I16 = mybir.dt.int16
I32 = mybir.dt.int32

B, I, H, NB = 8192, 1024, 1024, 14
MEM = 2**NB
NCORES = 8
BL = B // NCORES  # 1024 batch rows per core
KC = H // 128  # 8 contraction chunks
MT = BL // 128  # 8 output row-tiles per core
EPS = 1e-5
WSCALE = 256.0
EPS_SC = EPS * WSCALE * WSCALE  # eps for the x256-scaled pre-activation
RSQRT_MAGIC = 0x5EF759DF  # 0x5f3759df - 0x00400000: seed for rsqrt(2*vh)

# const_t packed layout (f32 columns)
C_CLIP = 0  # [2, 2] idx clip bounds
C_NEGMB = 2  # [28, 1] -Mb
C_PW = 3  # [28, 1] powers of two as bf16 pair-packed in f32
C_MAGIC = 4  # [128, 1] rsqrt seed magic (int32 bits)
C_IDENT = 5  # [128, 64] 128x128 bf16 identity (bitcast)
NCONST = 69

_CACHE = {}


def _build(trivial_gb: bool):
    """Trace the Bass/Tile module (shared by all 8 cores, SPMD)."""
    nc = bacc.Bacc(
        "TRN2", target_bir_lowering=False, debug=False, enable_asserts=True
    )

    x8_t = nc.dram_tensor("x8_t", [128, KC, BL], F8E4, kind="ExternalInput").ap()
    h8_t = nc.dram_tensor("h8_t", [128, KC, BL], F8E4, kind="ExternalInput").ap()
    h16_t = nc.dram_tensor("h16_t", [128, KC, BL], F16, kind="ExternalInput").ap()
    mw_t = nc.dram_tensor("mw_t", [128, KC, 2 * NB], F16, kind="ExternalInput").ap()
    # weights, [src, feat_in(part), feat_in(chunk), feat_out]; W,U fp8 / Qr,Ql bf16
    w8_t = nc.dram_tensor("w8_t", [2, 128, KC, H], F8E4, kind="ExternalInput").ap()
    wq_t = nc.dram_tensor("wq_t", [2, 128, KC, H], BF16, kind="ExternalInput").ap()
    bias16_t = nc.dram_tensor("bias16_t", [1, H], BF16, kind="ExternalInput").ap()
    const_t = nc.dram_tensor("const_t", [128, NCONST], F32, kind="ExternalInput").ap()
    mem_t = nc.dram_tensor("mem_t", [MEM, H], BF16, kind="ExternalInput").ap()
    if not trivial_gb:
        gam_t = nc.dram_tensor("gam_t", [128, H], F32, kind="ExternalInput").ap()
        bet_t = nc.dram_tensor("bet_t", [128, H], F32, kind="ExternalInput").ap()
    out_t = nc.dram_tensor("out_t", [BL, H], BF16, kind="ExternalOutput").ap()

    with tile.TileContext(nc) as tc:
        with ExitStack() as ctx:
            # ---------------- pools ----------------
            cpool = ctx.enter_context(tc.tile_pool(name="consts", bufs=1))
            apool = ctx.enter_context(tc.tile_pool(name="acts", bufs=1))
            hpool = ctx.enter_context(tc.tile_pool(name="h16", bufs=4))
            gpool = ctx.enter_context(tc.tile_pool(name="gathered", bufs=1))
            spool = ctx.enter_context(tc.tile_pool(name="small", bufs=2))
            epool = ctx.enter_context(tc.tile_pool(name="epilogue", bufs=2))
            # PSUM: tag "acc" rotates 3 two-bank slots (logits shares the
            # ring); tag "sm" rotates 2 one-bank slots (idx halves + PE
            # transposes). 3*2 + 2*1 = 8 banks.
            pp = ctx.enter_context(tc.tile_pool(name="psum", bufs=1, space="PSUM"))

            # gpsimd ucode library containing DMAGatherAnt; load it up front
            # so the Q7 IRAM reload overlaps the initial DMAs.
            nc.gpsimd.load_library(library_config.attnmlp)

            # ---------------- input loads (issue order = priority) ----------
            mw_sb = cpool.tile([128, KC, 2 * NB], F16, tag="mw")
            nc.sync.dma_start(mw_sb[:], mw_t[:])
            const_sb = cpool.tile([128, NCONST], F32, tag="const")
            nc.sync.dma_start(const_sb[:], const_t[:])
            clip_sb = const_sb[0:2, C_CLIP : C_CLIP + 2]
            negmb_sb = const_sb[0 : 2 * NB, C_NEGMB : C_NEGMB + 1]
            pw_sb = const_sb[0 : 2 * NB, C_PW : C_PW + 1].bitcast(BF16)
            magic_sb = const_sb[:, C_MAGIC : C_MAGIC + 1].bitcast(I32)
            ident_sb = const_sb[:, C_IDENT : C_IDENT + 64].bitcast(BF16)

            # warm the Sigmoid activation table while DMAs run (the only
            # ACT table in this kernel -> one table load total)
            warm = cpool.tile([128, 1], F32, tag="warm")
            nc.vector.memset(warm[:], 0.0)
            nc.scalar.activation(
                warm[:], warm[:], mybir.ActivationFunctionType.Sigmoid
            )
            ones_sb = cpool.tile([1, 128], BF16, tag="ones")
            nc.vector.memset(ones_sb[:], 1.0)

            # h16 for logits in 4 pieces, interleaved with x8/w0 so the
            # first xh matmuls aren't starved behind the full logits load
            h16p = []
            for piece in range(4):
                hp = hpool.tile([128, 2, BL], F16, tag="slab")
                h16p.append(hp)
            x8_sb = apool.tile([128, KC, BL], F8E4, tag="x8")
            h8_sb = apool.tile([128, KC, BL], F8E4, tag="h8")
            w_sb = [
                cpool.tile([128, KC, H], F8E4, tag=f"w{s}", name=f"w{s}")
                for s in range(2)
            ] + [
                cpool.tile([128, KC, H], BF16, tag=f"w{s}", name=f"wq{s}")
                for s in (2, 3)
            ]
            bias16_sb = cpool.tile([1, H], BF16, tag="bias16")

            for piece in range(4):
                nc.sync.dma_start(
                    h16p[piece][:], h16_t[:, 2 * piece : 2 * piece + 2, :]
                )
            nc.sync.dma_start(bias16_sb[:], bias16_t[:])
            # x and weights follow in k-chunk pieces so the xh matmuls can
            # start on partial data right after logits
            nc.sync.dma_start(x8_sb[:, 0:4, :], x8_t[:, 0:4, :])
            nc.sync.dma_start(x8_sb[:, 4:8, :], x8_t[:, 4:8, :])
            for kk in range(2):
                nc.sync.dma_start(
                    w_sb[0][:, 4 * kk : 4 * kk + 4, :],
                    w8_t[0, :, 4 * kk : 4 * kk + 4, :],
                )
            nc.sync.dma_start(h8_sb[:, 0:4, :], h8_t[:, 0:4, :])
            nc.sync.dma_start(h8_sb[:, 4:8, :], h8_t[:, 4:8, :])
            for kk in range(2):
                nc.sync.dma_start(
                    w_sb[1][:, 4 * kk : 4 * kk + 4, :],
                    w8_t[1, :, 4 * kk : 4 * kk + 4, :],
                )
            for s in (2, 3):
                for kk in range(4):
                    nc.sync.dma_start(
                        w_sb[s][:, 2 * kk : 2 * kk + 2, :],
                        wq_t[s - 2, :, 2 * kk : 2 * kk + 2, :],
                    )
            if not trivial_gb:
                gam_sb = cpool.tile([128, H], F32, tag="gam")
                nc.sync.dma_start(gam_sb[:], gam_t[:])
                bet_sb = cpool.tile([128, H], F32, tag="bet")
                nc.sync.dma_start(bet_sb[:], bet_t[:])
                zero_sb = cpool.tile([128, 1], F32, tag="zero")
                nc.vector.memset(zero_sb[:], 0.0)

            def h16_chunk(k):
                return h16p[k // 2][:, k % 2, :]

            # ---------------- index pipeline ----------------
            # logits.T [28, BL] fp16 inputs, fp32 PSUM. Bank-outer (batch
            # halves): bank 0's bits/idx/wrap/gathers launch while bank 1's
            # logits matmuls are still streaming.
            logit_ps = pp.tile([2 * NB, BL], F32, tag="acc", bufs=3)
            bits_sb = spool.tile([2 * NB, BL], BF16, tag="bits")
            idx16 = spool.tile([2, BL], I16, tag="idx16")
            stg_r = []
            idxw_r = []
            for r in range(2):
                stg = spool.tile([32, 64], I16, tag="stage", name=f"stg{r}")
                stg_r.append(stg)
                idxw = spool.tile([128, 64], I16, tag="idxw", name=f"idxw{r}")
                idxw_r.append(idxw)
            HB = BL // 2
            g2_tiles = [[None, None], [None, None]]

            for n in range(2):
                sl = slice(n * 512, (n + 1) * 512)
                for k in range(KC):
                    nc.tensor.matmul(
                        logit_ps[:, sl],
                        mw_sb[:, k, :],
                        h16_chunk(k)[:, sl],
                        start=(k == 0),
                        stop=(k == KC - 1),
                    )
                # bits = (h@Mw.T + Mb > 0)  <=>  (h@Mw.T > -Mb), as 1.0/0.0
                nc.vector.tensor_scalar(
                    bits_sb[:, sl], logit_ps[:, sl], negmb_sb[:, 0:1], None,
                    mybir.AluOpType.is_gt,
                )
                # raw indices via tiny matmul with powers of two: [2, 512]
                idx_ps = pp.tile([2, 512], F32, tag="sm", bufs=2)
                nc.tensor.matmul(
                    idx_ps[:], pw_sb, bits_sb[:, sl], start=True, stop=True
                )
                # clip + cast to int16; per-partition clip bounds:
                # row0 -> [0, 8191], row1 -> [8192, 16383]
                nc.vector.tensor_scalar(
                    idx16[:, sl], idx_ps[:],
                    clip_sb[:, 0:1], clip_sb[:, 1:2],
                    mybir.AluOpType.max, mybir.AluOpType.min,
                )
                # wrap this batch-half into the [16-group, 32] gather layout:
                # stage S[i, 32n+q'] = idx[(32n+i)*16+q'%16] via a strided
                # DMA (16 cols, duplicated), then 32x32 DVE transposes into
                # idxw columns [32n:32n+32]
                for r in range(2):
                    stg = stg_r[r]
                    stg_j = stg[0:32, :].rearrange("p (j hq) -> p j hq", j=2)
                    with nc.allow_non_contiguous_dma(
                        reason="tiny idx wrap staging"
                    ):
                        nc.scalar.dma_start(
                            stg[0:32, 32 * n : 32 * n + 16],
                            idx16[r : r + 1, sl].rearrange(
                                "p (a b) -> p a b", b=16
                            ),
                        )
                    nc.vector.tensor_copy(
                        stg_j[:, n, 16:32], stg_j[:, n, 0:16]
                    )
                    for g in range(4):
                        nc.vector.transpose(
                            idxw_r[r][32 * g : 32 * (g + 1),
                                      32 * n : 32 * n + 32],
                            stg[:, 32 * n : 32 * n + 32],
                        )
                # launch this half's gathers immediately:
                # g2[r][n][p, c, :] = mem[idx_{(4*n+c)*128+p}, :] (bf16 rows)
                for r in range(2):
                    g2 = gpool.tile(
                        [128, HB // 128, H], BF16, tag=f"g2_{r}{n}",
                        name=f"g2_{r}{n}",
                    )
                    nc.gpsimd.dma_gather(
                        out_ap=g2[:],
                        in_ap=mem_t[:],
                        idxs_ap=idxw_r[r][:, n * 32 : (n + 1) * 32],
                        num_idxs=HB,
                        num_idxs_reg=HB,
                        elem_size=H,
                        transpose=False,
                    )
                    g2_tiles[r][n] = g2

            # ---------------- main matmuls + epilogue ----------------
            srcs_xh = [(x8_sb, 0), (h8_sb, 1)]
            ps_tiles = {}
            mem_sb = [[None] * MT, [None] * MT]

            def emit_transpose(c):
                # PE-transpose gathered rows of batch-block c into
                # [feat, batch] bf16 tiles; PSUM->SBUF copies alternate
                # DVE / ACT so neither engine rate-limits the pipeline
                for r in range(2):
                    g2 = g2_tiles[r][c // 4]
                    cc = c % 4
                    mt = gpool.tile([128, KC, 128], BF16, tag=f"mem{r}_{c}")
                    for k in range(KC):
                        tp = pp.tile([128, 128], BF16, tag="sm", bufs=2)
                        nc.tensor.transpose(
                            tp[:], g2[:, cc, k * 128 : (k + 1) * 128],
                            ident_sb[:],
                        )
                        if k % 2 == 0:
                            nc.vector.tensor_copy(mt[:, k, :], tp[:])
                        else:
                            nc.scalar.activation(
                                mt[:, k, :], tp[:],
                                mybir.ActivationFunctionType.Identity,
                            )
                    mem_sb[r][c] = mt

            def emit_xh(m):
                ps = pp.tile([128, H], F32, tag="acc", bufs=3)
                ps_tiles[m] = ps
                ms = slice(m * 128, (m + 1) * 128)
                # rank-1 bias matmul opens each bank's accumulation group
                for n in range(H // 512):
                    nc.tensor.matmul(
                        ps[:, n * 512 : (n + 1) * 512],
                        ones_sb[:],
                        bias16_sb[:, n * 512 : (n + 1) * 512],
                        start=True,
                        stop=False,
                    )
                # x/h terms in fp8 DoubleRow: two K-planes per
                # instruction, interleaved with the bf16 mem phases so the
                # activity governor stays quiet
                for si, (act, wi) in enumerate(srcs_xh):
                    for kp in range(KC // 2):
                        lhs = act[:, 2 * kp : 2 * kp + 2, ms]
                        for n in range(H // 512):
                            nc.tensor.matmul(
                                ps[:, n * 512 : (n + 1) * 512],
                                lhs,
                                w_sb[wi][:, 2 * kp : 2 * kp + 2,
                                         n * 512 : (n + 1) * 512],
                                start=False,
                                stop=False,
                                perf_mode=mybir.MatmulPerfMode.DoubleRow,
                            )

            def emit_mem_epilogue(m):
                ps = ps_tiles.pop(m)
                ms = slice(m * 128, (m + 1) * 128)
                st6 = epool.tile([128, 2, 6], F32, tag="st6")
                # bank-outer: bank 0 stops (and its bn_stats runs) while
                # bank 1's matmuls are still streaming
                for n in range(H // 512):
                    for si in range(2):
                        mt = mem_sb[si][m]  # [128, KC, 128] block for this m
                        for k in range(KC):
                            nc.tensor.matmul(
                                ps[:, n * 512 : (n + 1) * 512],
                                mt[:, k, :],
                                w_sb[2 + si][:, k, n * 512 : (n + 1) * 512],
                                start=False,
                                stop=(si == 1 and k == KC - 1),
                            )
                    nc.vector.bn_stats(
                        st6[:, n, :], ps[:, n * 512 : (n + 1) * 512]
                    )
                mv = epool.tile([128, 2], F32, tag="mv")
                nc.vector.bn_aggr(mv[:], st6.rearrange("p a b -> p (a b)"))
                # rstd = 1/sqrt(var+eps) entirely on DVE:
                # quake seed from vh=(var+eps)/2 bits, then 2 Newton steps
                # y <- y*(1.5 - vh*y^2).
                st = epool.tile([128, 4], F32, tag="rs")
                vh = st[:, 0:1]
                y = st[:, 1:2]
                a_ = st[:, 2:3]
                nmu = st[:, 3:4]
                nc.vector.tensor_scalar(
                    vh, mv[:, 1:2], 0.5, EPS_SC * 0.5,
                    mybir.AluOpType.mult, mybir.AluOpType.add,
                )
                nc.vector.tensor_scalar(
                    a_.bitcast(I32), vh.bitcast(I32), 1, None,
                    mybir.AluOpType.logical_shift_right,
                )
                nc.vector.tensor_tensor(
                    y.bitcast(I32), magic_sb[:], a_.bitcast(I32),
                    mybir.AluOpType.subtract,
                )
                for _ in range(2):
                    nc.vector.tensor_tensor(a_, y, y, mybir.AluOpType.mult)
                    nc.vector.tensor_tensor(a_, a_, vh, mybir.AluOpType.mult)
                    nc.vector.tensor_scalar(
                        a_, a_, 1.5, -1.0,
                        mybir.AluOpType.subtract, mybir.AluOpType.mult,
                    )
                    nc.vector.tensor_tensor(y, y, a_, mybir.AluOpType.mult)
                nc.vector.tensor_scalar(
                    nmu, mv[:, 0:1], y, -1.0,
                    mybir.AluOpType.mult, mybir.AluOpType.mult,
                )
                o = epool.tile([128, H], BF16, tag="o")
                if trivial_gb:
                    # out = sigmoid((t - mu) * rstd), read from PSUM
                    nc.scalar.activation(
                        o[:], ps[:], mybir.ActivationFunctionType.Sigmoid,
                        bias=nmu, scale=y,
                    )
                else:
                    xh = epool.tile([128, H], F32, tag="xh")
                    nc.scalar.activation(
                        xh[:], ps[:], mybir.ActivationFunctionType.Identity,
                        bias=nmu, scale=y,
                    )
                    nc.vector.tensor_tensor(
                        xh[:], xh[:], gam_sb[:], mybir.AluOpType.mult
                    )
                    nc.vector.tensor_tensor(
                        xh[:], xh[:], bet_sb[:], mybir.AluOpType.add
                    )
                    nc.scalar.activation(
                        o[:], xh[:], mybir.ActivationFunctionType.Sigmoid,
                        bias=zero_sb[:, 0:1],
                    )
                nc.sync.dma_start(out_t[ms, :], o[:])

            # "acc" slot ring: L(s0) x0(s1) x1(s2) x2(s0) x3(s1) x4(s2)
            # x5(s0) x6(s1) x7(s2); each reuse is covered by >=1 emitted
            # mem-tile of PE work so the epilogue chain never stalls the PE.
            emit_xh(0)
            emit_xh(1)
            emit_xh(2)
            for c in range(4):
                emit_transpose(c)
            emit_mem_epilogue(0)
            emit_mem_epilogue(1)
            emit_xh(3)
            emit_xh(4)
            for c in range(4, 8):
                emit_transpose(c)
            emit_mem_epilogue(2)
            emit_mem_epilogue(3)
            emit_xh(5)
            emit_xh(6)
            emit_mem_epilogue(4)
            emit_mem_epilogue(5)
            emit_xh(7)
            emit_mem_epilogue(6)
            emit_mem_epilogue(7)

    nc.compile()  # bacc register allocation / DCE
    return nc


def _to_kxp(a, dtype):
    """[batch, feat] -> [128, KC, batch] with feat = k*128 + p."""
    t = np.ascontiguousarray(a.T.reshape(KC, 128, -1).transpose(1, 0, 2))
    return t.astype(dtype)


def prep(inputs):
    """Host-side shard/layout prep. Returns (in_maps, trivial_gb)."""
    x = np.asarray(inputs["x"], np.float32)
    h = np.asarray(inputs["h_prev"], np.float32)
    memory = np.asarray(inputs["memory"], np.float32)
    gamma = np.asarray(inputs["gamma"], np.float32)
    beta = np.asarray(inputs["beta"], np.float32)
    trivial_gb = bool(np.all(gamma == 1.0) and np.all(beta == 0.0))

    bf = ml_dtypes.bfloat16
    e4 = ml_dtypes.float8_e4m3
    # W is [out, in]; the kernel wants w[p, k, n] = W[n, k*128+p], which is
    # exactly _to_kxp applied to W with (out, in) in the (batch, feat) slots.
    w8_cat = np.stack(
        [
            _to_kxp(np.asarray(inputs[n], np.float32) * WSCALE, e4)
            for n in ("Ww", "Uw")
        ]
    )
    wq_cat = np.stack(
        [
            _to_kxp(np.asarray(inputs[n], np.float32) * WSCALE, bf)
            for n in ("Qrw", "Qlw")
        ]
    )
    mw = _to_kxp(np.asarray(inputs["Mw"], np.float32), np.float16)

    pw = np.zeros((2 * NB, 2), np.float32)
    pw[:NB, 0] = 2.0 ** np.arange(NB - 1, -1, -1)
    pw[NB:, 1] = 2.0 ** np.arange(NB - 1, -1, -1)
    clip = np.array(
        [[0.0, MEM // 2 - 1], [MEM // 2, MEM - 1]], np.float32
    )  # [row, (lo, hi)]

    mem16 = memory.astype(bf)
    bias16 = (
        (
            np.asarray(inputs["Wb"], np.float32)
            + np.asarray(inputs["Ub"], np.float32)
            + np.asarray(inputs["Qrb"], np.float32)
            + np.asarray(inputs["Qlb"], np.float32)
        )
        * WSCALE
    ).astype(bf).reshape(1, H)

    # pack the small constants into one [128, NCONST] f32 buffer
    const = np.zeros((128, NCONST), np.float32)
    const[:2, C_CLIP : C_CLIP + 2] = clip
    const[: 2 * NB, C_NEGMB : C_NEGMB + 1] = -np.asarray(
        inputs["Mb"], np.float32
    ).reshape(2 * NB, 1)
    const[: 2 * NB, C_PW : C_PW + 1] = pw.astype(bf).view(np.float32)
    const[:, C_MAGIC : C_MAGIC + 1] = (
        np.full((128, 1), RSQRT_MAGIC, np.int32).view(np.float32)
    )
    ident16 = np.eye(128, dtype=np.float32).astype(bf)
    const[:, C_IDENT : C_IDENT + 64] = ident16.view(np.float32)

    common = dict(
        w8_t=w8_cat, wq_t=wq_cat, bias16_t=bias16, const_t=const,
        mem_t=mem16, mw_t=mw,
    )
    if not trivial_gb:
        common["gam_t"] = np.ascontiguousarray(np.broadcast_to(gamma, (128, H)))
        common["bet_t"] = np.ascontiguousarray(np.broadcast_to(beta, (128, H)))

    in_maps = []
    for c in range(NCORES):
        xs = x[c * BL : (c + 1) * BL]
        hs = h[c * BL : (c + 1) * BL]
        in_maps.append(
            dict(
                x8_t=_to_kxp(xs, e4),
                h8_t=_to_kxp(hs, e4),
                h16_t=_to_kxp(hs, np.float16),
                **common,
            )
        )
    return in_maps, trivial_gb


def get_nc(trivial_gb):
    key = ("nc", trivial_gb)
    if key not in _CACHE:
        _CACHE[key] = _build(trivial_gb)
    return _CACHE[key]


def run(inputs, trace=False, **kw):
    in_maps, trivial_gb = prep(inputs)
    nc = get_nc(trivial_gb)
    res = run_bass_kernel_spmd(
        nc, in_maps, core_ids=list(range(NCORES)), trace=trace, **kw
    )
    out = np.concatenate([res.results[c]["out_t"] for c in range(NCORES)], axis=0)
    return out.astype(np.float32), res


def kernel(**inputs):
    return run(inputs)[0]


# revision 51
# speedup vs baseline: 1.3529x; 1.0061x over previous
"""Trainium2 Bass kernel for nn_BinaryMemoryRNN (scatter_memory).

Computation (reference):
    logits = h_prev @ Mw.T + Mb                 # [B, 28]
    b1/b2  = bits of logits halves (> 0)
    idx1   = clip(sum(b1 * 2^(13-j)), 0, 8191)
    idx2   = clip(sum(b2 * 2^(13-j)), 8192, 16383)
    pre    = x @ Ww.T + h_prev @ Uw.T + mem[idx1] @ Qrw.T + mem[idx2] @ Qlw.T + bias
    out    = sigmoid(layernorm(pre) * gamma + beta)

Strategy: data-parallel over batch across 8 cores (1024 rows each).
  - x/h activations in fp8-e4m3 as the stationary matmul operand; weights
    stream as bf16 (the fp8xfp8 combination runs at half rate on TRN2, and
    fp8 as the moving operand is slow - fp8-stationary x bf16-moving is
    full rate). Weights scaled x256 so fp8 activations meet them in a
    range the layernorm renormalizes away; only the bias needs the x256.
  - logits matmul in fp16 (sign-sensitive index bits: fp16's 10 mantissa
    bits flip ~1e-3 of them; full fp32 would cost 4x the PE time and 2x
    the critical-path DMA).
  - memory table replicated in DRAM as bf16 [16384, 1024]; rows fetched
    with gpsimd.dma_gather (row layout), PE-transposed to [feat, batch].
    PSUM->SBUF copies of transposed tiles alternate DVE / ACT engines.
  - bias enters the PSUM accumulation as a rank-1 (ones x bias) matmul,
    so the epilogue reads layernorm stats straight from PSUM; mem matmuls
    are emitted bank-outer so bank-0 stats overlap bank-1 matmuls.
  - rstd = 1/sqrt(var+eps) via DVE quake-seed + 2 Newton steps: the ACT
    engine's activation table never leaves Sigmoid. Output written bf16.
  - DMA issue order doubles as the priority schedule: logits path first
    (mw, h16 pieces), then x/w0 for the first matmuls, then the rest.
"""

import sys

sys.path.insert(0, "/opt/trn_rl_repo")

from contextlib import ExitStack

import numpy as np
import ml_dtypes

import concourse.bass as bass
import concourse.tile as tile
from concourse import bacc, mybir, library_config
from concourse.bass_utils import run_bass_kernel_spmd

F32 = mybir.dt.float32
F16 = mybir.dt.float16
BF16 = mybir.dt.bfloat16
F8E4 = mybir.dt.float8e4
I16 = mybir.dt.int16
I32 = mybir.dt.int32

B, I, H, NB = 8192, 1024, 1024, 14
MEM = 2**NB
NCORES = 8
BL = B // NCORES  # 1024 batch rows per core
KC = H // 128  # 8 contraction chunks
MT = BL // 128  # 8 output row-tiles per core
EPS = 1e-5
WSCALE = 256.0
EPS_SC = EPS * WSCALE * WSCALE  # eps for the x256-scaled pre-activation
RSQRT_MAGIC = 0x5EF759DF  # 0x5f3759df - 0x00400000: seed for rsqrt(2*vh)

# const_t packed layout (f32 columns)
C_CLIP = 0  # [2, 2] idx clip bounds
C_NEGMB = 2  # [28, 1] -Mb
C_PW = 3  # [28, 1] powers of two as bf16 pair-packed in f32
C_MAGIC = 4  # [128, 1] rsqrt seed magic (int32 bits)
C_IDENT = 5  # [128, 64] 128x128 bf16 identity (bitcast)
NCONST = 69

_CACHE = {}


def _build(trivial_gb: bool):
    """Trace the Bass/Tile module (shared by all 8 cores, SPMD)."""
    nc = bacc.Bacc(
        "TRN2", target_bir_lowering=False, debug=False, enable_asserts=True
    )

    x8_t = nc.dram_tensor("x8_t", [128, KC, BL], F8E4, kind="ExternalInput").ap()
    h8_t = nc.dram_tensor("h8_t", [128, KC, BL], F8E4, kind="ExternalInput").ap()
    h16_t = nc.dram_tensor("h16_t", [128, KC, BL], F16, kind="ExternalInput").ap()
    mw_t = nc.dram_tensor("mw_t", [128, KC, 2 * NB], F16, kind="ExternalInput").ap()
    # weights, [src, feat_in(part), feat_in(chunk), feat_out]; W,U fp8 / Qr,Ql bf16
    w8_t = nc.dram_tensor("w8_t", [2, 128, KC, H], F8E4, kind="ExternalInput").ap()
    wq_t = nc.dram_tensor("wq_t", [2, 128, KC, H], BF16, kind="ExternalInput").ap()
    bias16_t = nc.dram_tensor("bias16_t", [1, H], BF16, kind="ExternalInput").ap()
    const_t = nc.dram_tensor("const_t", [128, NCONST], F32, kind="ExternalInput").ap()
    mem_t = nc.dram_tensor("mem_t", [MEM, H], BF16, kind="ExternalInput").ap()
    if not trivial_gb:
        gam_t = nc.dram_tensor("gam_t", [128, H], F32, kind="ExternalInput").ap()
        bet_t = nc.dram_tensor("bet_t", [128, H], F32, kind="ExternalInput").ap()
    out_t = nc.dram_tensor("out_t", [BL, H], BF16, kind="ExternalOutput").ap()

    with tile.TileContext(nc) as tc:
        with ExitStack() as ctx:
            # ---------------- pools ----------------
            cpool = ctx.enter_context(tc.tile_pool(name="consts", bufs=1))
            apool = ctx.enter_context(tc.tile_pool(name="acts", bufs=1))
            hpool = ctx.enter_context(tc.tile_pool(name="h16", bufs=1))
            gpool = ctx.enter_context(tc.tile_pool(name="gathered", bufs=1))
            spool = ctx.enter_context(tc.tile_pool(name="small", bufs=2))
            epool = ctx.enter_context(tc.tile_pool(name="epilogue", bufs=2))
            # PSUM: tag "acc" rotates 3 two-bank slots (logits shares the
            # ring); tag "sm" rotates 2 one-bank slots (idx halves + PE
            # transposes). 3*2 + 2*1 = 8 banks.
            pp = ctx.enter_context(tc.tile_pool(name="psum", bufs=1, space="PSUM"))

            # gpsimd ucode library containing DMAGatherAnt; load it up front
            # so the Q7 IRAM reload overlaps the initial DMAs.
            nc.gpsimd.load_library(library_config.attnmlp)

            # ---------------- input loads (issue order = priority) ----------
            mw_sb = cpool.tile([128, KC, 2 * NB], F16, tag="mw")
            nc.sync.dma_start(mw_sb[:], mw_t[:])
            const_sb = cpool.tile([128, NCONST], F32, tag="const")
            nc.sync.dma_start(const_sb[:], const_t[:])
            clip_sb = const_sb[0:2, C_CLIP : C_CLIP + 2]
            negmb_sb = const_sb[0 : 2 * NB, C_NEGMB : C_NEGMB + 1]
            pw_sb = const_sb[0 : 2 * NB, C_PW : C_PW + 1].bitcast(BF16)
            magic_sb = const_sb[:, C_MAGIC : C_MAGIC + 1].bitcast(I32)
            ident_sb = const_sb[:, C_IDENT : C_IDENT + 64].bitcast(BF16)

            # warm the Sigmoid activation table while DMAs run (the only
            # ACT table in this kernel -> one table load total)
            warm = cpool.tile([128, 1], F32, tag="warm")
            nc.vector.memset(warm[:], 0.0)
            nc.scalar.activation(
                warm[:], warm[:], mybir.ActivationFunctionType.Sigmoid
            )
            ones_sb = cpool.tile([1, 128], BF16, tag="ones")
            nc.vector.memset(ones_sb[:], 1.0)

            # h16 for logits, loaded in two batch-half DMAs (all k
            # chunks each) so bank 0's logits/idx/gathers launch after 1.1MB
            h16_sb = hpool.tile([128, KC, BL], F16, tag="h16")
            x8_sb = apool.tile([128, KC, BL], F8E4, tag="x8")
            h8_sb = apool.tile([128, KC, BL], F8E4, tag="h8")
            w_sb = [
                cpool.tile([128, KC, H], F8E4, tag=f"w{s}", name=f"w{s}")
                for s in range(2)
            ] + [
                cpool.tile([128, KC, H], BF16, tag=f"w{s}", name=f"wq{s}")
                for s in (2, 3)
            ]
            bias16_sb = cpool.tile([1, H], BF16, tag="bias16")

            with nc.allow_non_contiguous_dma(reason="h16 batch-half load"):
                for n in range(2):
                    nc.sync.dma_start(
                        h16_sb[:, :, n * 512 : (n + 1) * 512],
                        h16_t[:, :, n * 512 : (n + 1) * 512],
                    )
            nc.sync.dma_start(bias16_sb[:], bias16_t[:])
            # x and weights follow in k-chunk pieces so the xh matmuls can
            # start on partial data right after logits
            nc.sync.dma_start(x8_sb[:, 0:4, :], x8_t[:, 0:4, :])
            nc.sync.dma_start(x8_sb[:, 4:8, :], x8_t[:, 4:8, :])
            for kk in range(2):
                nc.sync.dma_start(
                    w_sb[0][:, 4 * kk : 4 * kk + 4, :],
                    w8_t[0, :, 4 * kk : 4 * kk + 4, :],
                )
            nc.sync.dma_start(h8_sb[:, 0:4, :], h8_t[:, 0:4, :])
            nc.sync.dma_start(h8_sb[:, 4:8, :], h8_t[:, 4:8, :])
            for kk in range(2):
                nc.sync.dma_start(
                    w_sb[1][:, 4 * kk : 4 * kk + 4, :],
                    w8_t[1, :, 4 * kk : 4 * kk + 4, :],
                )
            for s in (2, 3):
                for kk in range(4):
                    nc.sync.dma_start(
                        w_sb[s][:, 2 * kk : 2 * kk + 2, :],
                        wq_t[s - 2, :, 2 * kk : 2 * kk + 2, :],
                    )
            if not trivial_gb:
                gam_sb = cpool.tile([128, H], F32, tag="gam")
                nc.sync.dma_start(gam_sb[:], gam_t[:])
                bet_sb = cpool.tile([128, H], F32, tag="bet")
                nc.sync.dma_start(bet_sb[:], bet_t[:])
                zero_sb = cpool.tile([128, 1], F32, tag="zero")
                nc.vector.memset(zero_sb[:], 0.0)

            def h16_chunk(k):
                return h16_sb[:, k, :]

            # ---------------- index pipeline ----------------
            # logits.T [28, BL] fp16 inputs, fp32 PSUM. Bank-outer (batch
            # halves): bank 0's bits/idx/wrap/gathers launch while bank 1's
            # logits matmuls are still streaming.
            logit_ps = pp.tile([2 * NB, BL], F32, tag="acc", bufs=3)
            bits_sb = spool.tile([2 * NB, BL], BF16, tag="bits")
            idx16 = spool.tile([2, BL], I16, tag="idx16")
            stg_r = []
            for r in range(2):
                stg = spool.tile([32, 64], I16, tag="stage", name=f"stg{r}")
                stg_r.append(stg)
            # combined wrapped-idx tile: cols [n*64 + r*32 : +32] hold the
            # r-tensor indices of batch-half n
            idxw = spool.tile([128, 128], I16, tag="idxw")
            HB = BL // 2
            g2_half = [None, None]

            for n in range(2):
                sl = slice(n * 512, (n + 1) * 512)
                for k in range(KC):
                    nc.tensor.matmul(
                        logit_ps[:, sl],
                        mw_sb[:, k, :],
                        h16_chunk(k)[:, sl],
                        start=(k == 0),
                        stop=(k == KC - 1),
                    )
                # bits = (h@Mw.T + Mb > 0)  <=>  (h@Mw.T > -Mb), as 1.0/0.0
                nc.vector.tensor_scalar(
                    bits_sb[:, sl], logit_ps[:, sl], negmb_sb[:, 0:1], None,
                    mybir.AluOpType.is_gt,
                )
                # raw indices via tiny matmul with powers of two: [2, 512]
                idx_ps = pp.tile([2, 512], F32, tag="sm", bufs=2)
                nc.tensor.matmul(
                    idx_ps[:], pw_sb, bits_sb[:, sl], start=True, stop=True
                )
                # clip + cast to int16; per-partition clip bounds:
                # row0 -> [0, 8191], row1 -> [8192, 16383]
                nc.vector.tensor_scalar(
                    idx16[:, sl], idx_ps[:],
                    clip_sb[:, 0:1], clip_sb[:, 1:2],
                    mybir.AluOpType.max, mybir.AluOpType.min,
                )
                # wrap this batch-half into the [16-group, 32] gather layout:
                # stage S[i, 32n+q'] = idx[(32n+i)*16+q'%16] via a strided
                # DMA (16 cols, duplicated), then 32x32 DVE transposes into
                # idxw columns [32n:32n+32]
                for r in range(2):
                    stg = stg_r[r]
                    stg_j = stg[0:32, :].rearrange("p (j hq) -> p j hq", j=2)
                    with nc.allow_non_contiguous_dma(
                        reason="tiny idx wrap staging"
                    ):
                        nc.scalar.dma_start(
                            stg[0:32, 32 * n : 32 * n + 16],
                            idx16[r : r + 1, sl].rearrange(
                                "p (a b) -> p a b", b=16
                            ),
                        )
                    nc.vector.tensor_copy(
                        stg_j[:, n, 16:32], stg_j[:, n, 0:16]
                    )
                    for g in range(4):
                        nc.vector.transpose(
                            idxw[32 * g : 32 * (g + 1),
                                 n * 64 + r * 32 : n * 64 + r * 32 + 32],
                            stg[:, 32 * n : 32 * n + 32],
                        )
                # one combined gather per batch-half (both mem tensors):
                # out blocks 0-3 = idx1 rows, 4-7 = idx2 rows
                g2 = gpool.tile(
                    [128, 2 * HB // 128, H], BF16, tag=f"g2_{n}",
                    name=f"g2_{n}",
                )
                nc.gpsimd.dma_gather(
                    out_ap=g2[:],
                    in_ap=mem_t[:],
                    idxs_ap=idxw[:, n * 64 : (n + 1) * 64],
                    num_idxs=2 * HB,
                    num_idxs_reg=2 * HB,
                    elem_size=H,
                    transpose=False,
                )
                g2_half[n] = g2

            # ---------------- main matmuls + epilogue ----------------
            srcs_xh = [(x8_sb, 0), (h8_sb, 1)]
            ps_tiles = {}
            mem_sb = [[None] * MT, [None] * MT]

            def emit_transpose(c):
                # PE-transpose gathered rows of batch-block c into
                # [feat, batch] bf16 tiles; PSUM->SBUF copies alternate
                # DVE / ACT so neither engine rate-limits the pipeline
                for r in range(2):
                    g2 = g2_half[c // 4]
                    cc = 4 * r + c % 4
                    mt = gpool.tile([128, KC, 128], BF16, tag=f"mem{r}_{c}")
                    for k in range(KC):
                        tp = pp.tile([128, 128], BF16, tag="sm", bufs=2)
                        nc.tensor.transpose(
                            tp[:], g2[:, cc, k * 128 : (k + 1) * 128],
                            ident_sb[:],
                        )
                        if k % 2 == 0:
                            nc.vector.tensor_copy(mt[:, k, :], tp[:])
                        else:
                            nc.scalar.activation(
                                mt[:, k, :], tp[:],
                                mybir.ActivationFunctionType.Identity,
                            )
                    mem_sb[r][c] = mt

            def emit_xh(m):
                ps = pp.tile([128, H], F32, tag="acc", bufs=3)
                ps_tiles[m] = ps
                ms = slice(m * 128, (m + 1) * 128)
                # rank-1 bias matmul opens each bank's accumulation group
                for n in range(H // 512):
                    nc.tensor.matmul(
                        ps[:, n * 512 : (n + 1) * 512],
                        ones_sb[:],
                        bias16_sb[:, n * 512 : (n + 1) * 512],
                        start=True,
                        stop=False,
                    )
                # x/h terms in fp8 DoubleRow: two K-planes per
                # instruction, interleaved with the bf16 mem phases so the
                # activity governor stays quiet
                for si, (act, wi) in enumerate(srcs_xh):
                    for kp in range(KC // 2):
                        lhs = act[:, 2 * kp : 2 * kp + 2, ms]
                        for n in range(H // 512):
                            nc.tensor.matmul(
                                ps[:, n * 512 : (n + 1) * 512],
                                lhs,
                                w_sb[wi][:, 2 * kp : 2 * kp + 2,
                                         n * 512 : (n + 1) * 512],
                                start=False,
                                stop=False,
                                perf_mode=mybir.MatmulPerfMode.DoubleRow,
                            )

            def emit_mem_epilogue(m):
                ps = ps_tiles.pop(m)
                ms = slice(m * 128, (m + 1) * 128)
                st6 = epool.tile([128, 2, 6], F32, tag="st6")
                # bank-outer: bank 0 stops (and its bn_stats runs) while
                # bank 1's matmuls are still streaming
                for n in range(H // 512):
                    for si in range(2):
                        mt = mem_sb[si][m]  # [128, KC, 128] block for this m
                        for k in range(KC):
                            nc.tensor.matmul(
                                ps[:, n * 512 : (n + 1) * 512],
                                mt[:, k, :],
                                w_sb[2 + si][:, k, n * 512 : (n + 1) * 512],
                                start=False,
                                stop=(si == 1 and k == KC - 1),
                            )
                    nc.vector.bn_stats(
                        st6[:, n, :], ps[:, n * 512 : (n + 1) * 512]
                    )
                mv = epool.tile([128, 2], F32, tag="mv")
                nc.vector.bn_aggr(mv[:], st6.rearrange("p a b -> p (a b)"))
                # rstd = 1/sqrt(var+eps) entirely on DVE:
                # quake seed from vh=(var+eps)/2 bits, then 2 Newton steps
                # y <- y*(1.5 - vh*y^2).
                st = epool.tile([128, 4], F32, tag="rs")
                vh = st[:, 0:1]
                y = st[:, 1:2]
                a_ = st[:, 2:3]
                nmu = st[:, 3:4]
                nc.vector.tensor_scalar(
                    vh, mv[:, 1:2], 0.5, EPS_SC * 0.5,
                    mybir.AluOpType.mult, mybir.AluOpType.add,
                )
                nc.vector.tensor_scalar(
                    a_.bitcast(I32), vh.bitcast(I32), 1, None,
                    mybir.AluOpType.logical_shift_right,
                )
                nc.vector.tensor_tensor(
                    y.bitcast(I32), magic_sb[:], a_.bitcast(I32),
                    mybir.AluOpType.subtract,
                )
                for _ in range(1):
                    nc.vector.tensor_tensor(a_, y, y, mybir.AluOpType.mult)
                    nc.vector.tensor_tensor(a_, a_, vh, mybir.AluOpType.mult)
                    nc.vector.tensor_scalar(
                        a_, a_, 1.5, -1.0,
                        mybir.AluOpType.subtract, mybir.AluOpType.mult,
                    )
                    nc.vector.tensor_tensor(y, y, a_, mybir.AluOpType.mult)
                nc.vector.tensor_scalar(
                    nmu, mv[:, 0:1], y, -1.0,
                    mybir.AluOpType.mult, mybir.AluOpType.mult,
                )
                o = epool.tile([128, H], BF16, tag="o")
                if trivial_gb:
                    # out = sigmoid((t - mu) * rstd), read from PSUM
                    nc.scalar.activation(
                        o[:], ps[:], mybir.ActivationFunctionType.Sigmoid,
                        bias=nmu, scale=y,
                    )
                else:
                    xh = epool.tile([128, H], F32, tag="xh")
                    nc.scalar.activation(
                        xh[:], ps[:], mybir.ActivationFunctionType.Identity,
                        bias=nmu, scale=y,
                    )
                    nc.vector.tensor_tensor(
                        xh[:], xh[:], gam_sb[:], mybir.AluOpType.mult
                    )
                    nc.vector.tensor_tensor(
                        xh[:], xh[:], bet_sb[:], mybir.AluOpType.add
                    )
                    nc.scalar.activation(
                        o[:], xh[:], mybir.ActivationFunctionType.Sigmoid,
                        bias=zero_sb[:, 0:1],
                    )
                nc.sync.dma_start(out_t[ms, :], o[:])

            # "acc" slot ring: L(s0) x0(s1) x1(s2) x2(s0) x3(s1) x4(s2)
            # x5(s0) x6(s1) x7(s2); each reuse is covered by >=1 emitted
            # mem-tile of PE work so the epilogue chain never stalls the PE.
            emit_xh(0)
            emit_xh(1)
            emit_xh(2)
            for c in range(4):
                emit_transpose(c)
            emit_mem_epilogue(0)
            emit_mem_epilogue(1)
            emit_xh(3)
            emit_xh(4)
            for c in range(4, 8):
                emit_transpose(c)
            emit_mem_epilogue(2)
            emit_mem_epilogue(3)
            emit_xh(5)
            emit_xh(6)
            emit_mem_epilogue(4)
            emit_mem_epilogue(5)
            emit_xh(7)
            emit_mem_epilogue(6)
            emit_mem_epilogue(7)

    nc.compile()  # bacc register allocation / DCE
    return nc


def _to_kxp(a, dtype):
    """[batch, feat] -> [128, KC, batch] with feat = k*128 + p."""
    t = np.ascontiguousarray(a.T.reshape(KC, 128, -1).transpose(1, 0, 2))
    return t.astype(dtype)


def prep(inputs):
    """Host-side shard/layout prep. Returns (in_maps, trivial_gb)."""
    x = np.asarray(inputs["x"], np.float32)
    h = np.asarray(inputs["h_prev"], np.float32)
    memory = np.asarray(inputs["memory"], np.float32)
    gamma = np.asarray(inputs["gamma"], np.float32)
    beta = np.asarray(inputs["beta"], np.float32)
    trivial_gb = bool(np.all(gamma == 1.0) and np.all(beta == 0.0))

    bf = ml_dtypes.bfloat16
    e4 = ml_dtypes.float8_e4m3
    # W is [out, in]; the kernel wants w[p, k, n] = W[n, k*128+p], which is
    # exactly _to_kxp applied to W with (out, in) in the (batch, feat) slots.
    w8_cat = np.stack(
        [
            _to_kxp(np.asarray(inputs[n], np.float32) * WSCALE, e4)
            for n in ("Ww", "Uw")
        ]
    )
    wq_cat = np.stack(
        [
            _to_kxp(np.asarray(inputs[n], np.float32) * WSCALE, bf)
            for n in ("Qrw", "Qlw")
        ]
    )
    mw = _to_kxp(np.asarray(inputs["Mw"], np.float32), np.float16)

    pw = np.zeros((2 * NB, 2), np.float32)
    pw[:NB, 0] = 2.0 ** np.arange(NB - 1, -1, -1)
    pw[NB:, 1] = 2.0 ** np.arange(NB - 1, -1, -1)
    clip = np.array(
        [[0.0, MEM // 2 - 1], [MEM // 2, MEM - 1]], np.float32
    )  # [row, (lo, hi)]

    mem16 = memory.astype(bf)
    bias16 = (
        (
            np.asarray(inputs["Wb"], np.float32)
            + np.asarray(inputs["Ub"], np.float32)
            + np.asarray(inputs["Qrb"], np.float32)
            + np.asarray(inputs["Qlb"], np.float32)
        )
        * WSCALE
    ).astype(bf).reshape(1, H)

    # pack the small constants into one [128, NCONST] f32 buffer
    const = np.zeros((128, NCONST), np.float32)
    const[:2, C_CLIP : C_CLIP + 2] = clip
    const[: 2 * NB, C_NEGMB : C_NEGMB + 1] = -np.asarray(
        inputs["Mb"], np.float32
    ).reshape(2 * NB, 1)
    const[: 2 * NB, C_PW : C_PW + 1] = pw.astype(bf).view(np.float32)
    const[:, C_MAGIC : C_MAGIC + 1] = (
        np.full((128, 1), RSQRT_MAGIC, np.int32).view(np.float32)
    )
    ident16 = np.eye(128, dtype=np.float32).astype(bf)
    const[:, C_IDENT : C_IDENT + 64] = ident16.view(np.float32)

    common = dict(
        w8_t=w8_cat, wq_t=wq_cat, bias16_t=bias16, const_t=const,
        mem_t=mem16, mw_t=mw,
    )
    if not trivial_gb:
        common["gam_t"] = np.ascontiguousarray(np.broadcast_to(gamma, (128, H)))
        common["bet_t"] = np.ascontiguousarray(np.broadcast_to(beta, (128, H)))

    in_maps = []
    for c in range(NCORES):
        xs = x[c * BL : (c + 1) * BL]
        hs = h[c * BL : (c + 1) * BL]
        in_maps.append(
            dict(
                x8_t=_to_kxp(xs, e4),
                h8_t=_to_kxp(hs, e4),
                h16_t=_to_kxp(hs, np.float16),
                **common,
            )
        )
    return in_maps, trivial_gb


def get_nc(trivial_gb):
    key = ("nc", trivial_gb)
    if key not in _CACHE:
        _CACHE[key] = _build(trivial_gb)
    return _CACHE[key]


def run(inputs, trace=False, **kw):
    in_maps, trivial_gb = prep(inputs)
    nc = get_nc(trivial_gb)
    res = run_bass_kernel_spmd(
        nc, in_maps, core_ids=list(range(NCORES)), trace=trace, **kw
    )
    out = np.concatenate([res.results[c]["out_t"] for c in range(NCORES)], axis=0)
    return out.astype(np.float32), res


def kernel(**inputs):
    return run(inputs)[0]


# revision 52
# speedup vs baseline: 1.4038x; 1.0376x over previous
"""Trainium2 Bass kernel for nn_BinaryMemoryRNN (scatter_memory).

Computation (reference):
    logits = h_prev @ Mw.T + Mb                 # [B, 28]
    b1/b2  = bits of logits halves (> 0)
    idx1   = clip(sum(b1 * 2^(13-j)), 0, 8191)
    idx2   = clip(sum(b2 * 2^(13-j)), 8192, 16383)
    pre    = x @ Ww.T + h_prev @ Uw.T + mem[idx1] @ Qrw.T + mem[idx2] @ Qlw.T + bias
    out    = sigmoid(layernorm(pre) * gamma + beta)

Strategy: data-parallel over batch across 8 cores (1024 rows each).
  - x/h activations in fp8-e4m3 as the stationary matmul operand; weights
    stream as bf16 (the fp8xfp8 combination runs at half rate on TRN2, and
    fp8 as the moving operand is slow - fp8-stationary x bf16-moving is
    full rate). Weights scaled x256 so fp8 activations meet them in a
    range the layernorm renormalizes away; only the bias needs the x256.
  - logits matmul in fp16 (sign-sensitive index bits: fp16's 10 mantissa
    bits flip ~1e-3 of them; full fp32 would cost 4x the PE time and 2x
    the critical-path DMA).
  - memory table replicated in DRAM as bf16 [16384, 1024]; rows fetched
    with gpsimd.dma_gather (row layout), PE-transposed to [feat, batch].
    PSUM->SBUF copies of transposed tiles alternate DVE / ACT engines.
  - bias enters the PSUM accumulation as a rank-1 (ones x bias) matmul,
    so the epilogue reads layernorm stats straight from PSUM; mem matmuls
    are emitted bank-outer so bank-0 stats overlap bank-1 matmuls.
  - rstd = 1/sqrt(var+eps) via DVE quake-seed + 2 Newton steps: the ACT
    engine's activation table never leaves Sigmoid. Output written bf16.
  - DMA issue order doubles as the priority schedule: logits path first
    (mw, h16 pieces), then x/w0 for the first matmuls, then the rest.
"""

import sys

sys.path.insert(0, "/opt/trn_rl_repo")

from contextlib import ExitStack

import numpy as np
import ml_dtypes

import concourse.bass as bass
import concourse.tile as tile
from concourse import bacc, mybir, library_config
from concourse.bass_utils import run_bass_kernel_spmd

F32 = mybir.dt.float32
F16 = mybir.dt.float16
BF16 = mybir.dt.bfloat16
F8E4 = mybir.dt.float8e4
I16 = mybir.dt.int16
I32 = mybir.dt.int32

B, I, H, NB = 8192, 1024, 1024, 14
MEM = 2**NB
NCORES = 8
BL = B // NCORES  # 1024 batch rows per core
KC = H // 128  # 8 contraction chunks
MT = BL // 128  # 8 output row-tiles per core
EPS = 1e-5
WSCALE = 256.0
EPS_SC = EPS * WSCALE * WSCALE  # eps for the x256-scaled pre-activation
RSQRT_MAGIC = 0x5EF759DF  # 0x5f3759df - 0x00400000: seed for rsqrt(2*vh)

# const_t packed layout (f32 columns)
C_CLIP = 0  # [2, 2] idx clip bounds
C_NEGMB = 2  # [28, 1] -Mb
C_PW = 3  # [28, 1] powers of two as bf16 pair-packed in f32
C_MAGIC = 4  # [128, 1] rsqrt seed magic (int32 bits)
C_IDENT = 5  # [128, 64] 128x128 bf16 identity (bitcast)
NCONST = 69

_CACHE = {}


def _build(trivial_gb: bool):
    """Trace the Bass/Tile module (shared by all 8 cores, SPMD)."""
    nc = bacc.Bacc(
        "TRN2", target_bir_lowering=False, debug=False, enable_asserts=True
    )

    x8_t = nc.dram_tensor("x8_t", [128, KC, BL], F8E4, kind="ExternalInput").ap()
    h8_t = nc.dram_tensor("h8_t", [128, KC, BL], F8E4, kind="ExternalInput").ap()
    h16_t = nc.dram_tensor("h16_t", [128, 2, KC, 512], F16, kind="ExternalInput").ap()
    mw_t = nc.dram_tensor("mw_t", [128, KC, 2 * NB], F16, kind="ExternalInput").ap()
    # weights, [src, feat_in(part), feat_in(chunk), feat_out]; W,U fp8 / Qr,Ql bf16
    w8_t = nc.dram_tensor("w8_t", [2, 128, KC, H], F8E4, kind="ExternalInput").ap()
    wq_t = nc.dram_tensor("wq_t", [2, 128, KC, H], BF16, kind="ExternalInput").ap()
    bias16_t = nc.dram_tensor("bias16_t", [1, H], BF16, kind="ExternalInput").ap()
    const_t = nc.dram_tensor("const_t", [128, NCONST], F32, kind="ExternalInput").ap()
    mem_t = nc.dram_tensor("mem_t", [MEM, H], BF16, kind="ExternalInput").ap()
    if not trivial_gb:
        gam_t = nc.dram_tensor("gam_t", [128, H], F32, kind="ExternalInput").ap()
        bet_t = nc.dram_tensor("bet_t", [128, H], F32, kind="ExternalInput").ap()
    out_t = nc.dram_tensor("out_t", [BL, H], BF16, kind="ExternalOutput").ap()

    with tile.TileContext(nc) as tc:
        with ExitStack() as ctx:
            # ---------------- pools ----------------
            cpool = ctx.enter_context(tc.tile_pool(name="consts", bufs=1))
            apool = ctx.enter_context(tc.tile_pool(name="acts", bufs=1))
            hpool = ctx.enter_context(tc.tile_pool(name="h16", bufs=1))
            gpool = ctx.enter_context(tc.tile_pool(name="gathered", bufs=1))
            spool = ctx.enter_context(tc.tile_pool(name="small", bufs=2))
            epool = ctx.enter_context(tc.tile_pool(name="epilogue", bufs=2))
            # PSUM: tag "acc" rotates 3 two-bank slots (logits shares the
            # ring); tag "sm" rotates 2 one-bank slots (idx halves + PE
            # transposes). 3*2 + 2*1 = 8 banks.
            pp = ctx.enter_context(tc.tile_pool(name="psum", bufs=1, space="PSUM"))

            # gpsimd ucode library containing DMAGatherAnt; load it up front
            # so the Q7 IRAM reload overlaps the initial DMAs.
            nc.gpsimd.load_library(library_config.attnmlp)

            # ---------------- input loads (issue order = priority) ----------
            mw_sb = cpool.tile([128, KC, 2 * NB], F16, tag="mw")
            nc.sync.dma_start(mw_sb[:], mw_t[:])
            const_sb = cpool.tile([128, NCONST], F32, tag="const")
            nc.sync.dma_start(const_sb[:], const_t[:])
            clip_sb = const_sb[0:2, C_CLIP : C_CLIP + 2]
            negmb_sb = const_sb[0 : 2 * NB, C_NEGMB : C_NEGMB + 1]
            pw_sb = const_sb[0 : 2 * NB, C_PW : C_PW + 1].bitcast(BF16)
            magic_sb = const_sb[:, C_MAGIC : C_MAGIC + 1].bitcast(I32)
            ident_sb = const_sb[:, C_IDENT : C_IDENT + 64].bitcast(BF16)

            # warm the Sigmoid activation table while DMAs run (the only
            # ACT table in this kernel -> one table load total)
            warm = cpool.tile([128, 1], F32, tag="warm")
            nc.vector.memset(warm[:], 0.0)
            nc.scalar.activation(
                warm[:], warm[:], mybir.ActivationFunctionType.Sigmoid
            )
            ones_sb = cpool.tile([1, 128], BF16, tag="ones")
            nc.vector.memset(ones_sb[:], 1.0)

            # h16 for logits, half-major [128, half, KC, 512] so each
            # batch-half is one contiguous DMA and bank 0's logits/idx/
            # gathers launch after 1.1MB
            h16_sb = hpool.tile([128, 2, KC, 512], F16, tag="h16")
            x8_sb = apool.tile([128, KC, BL], F8E4, tag="x8")
            h8_sb = apool.tile([128, KC, BL], F8E4, tag="h8")
            w_sb = [
                cpool.tile([128, KC, H], F8E4, tag=f"w{s}", name=f"w{s}")
                for s in range(2)
            ] + [
                cpool.tile([128, KC, H], BF16, tag=f"w{s}", name=f"wq{s}")
                for s in (2, 3)
            ]
            bias16_sb = cpool.tile([1, H], BF16, tag="bias16")

            for n in range(2):
                nc.sync.dma_start(h16_sb[:, n], h16_t[:, n])
            nc.sync.dma_start(bias16_sb[:], bias16_t[:])
            # x and weights follow in k-chunk pieces so the xh matmuls can
            # start on partial data right after logits
            nc.sync.dma_start(x8_sb[:, 0:4, :], x8_t[:, 0:4, :])
            nc.sync.dma_start(x8_sb[:, 4:8, :], x8_t[:, 4:8, :])
            for kk in range(2):
                nc.sync.dma_start(
                    w_sb[0][:, 4 * kk : 4 * kk + 4, :],
                    w8_t[0, :, 4 * kk : 4 * kk + 4, :],
                )
            nc.sync.dma_start(h8_sb[:, 0:4, :], h8_t[:, 0:4, :])
            nc.sync.dma_start(h8_sb[:, 4:8, :], h8_t[:, 4:8, :])
            for kk in range(2):
                nc.sync.dma_start(
                    w_sb[1][:, 4 * kk : 4 * kk + 4, :],
                    w8_t[1, :, 4 * kk : 4 * kk + 4, :],
                )
            for s in (2, 3):
                for kk in range(4):
                    nc.sync.dma_start(
                        w_sb[s][:, 2 * kk : 2 * kk + 2, :],
                        wq_t[s - 2, :, 2 * kk : 2 * kk + 2, :],
                    )
            if not trivial_gb:
                gam_sb = cpool.tile([128, H], F32, tag="gam")
                nc.sync.dma_start(gam_sb[:], gam_t[:])
                bet_sb = cpool.tile([128, H], F32, tag="bet")
                nc.sync.dma_start(bet_sb[:], bet_t[:])
                zero_sb = cpool.tile([128, 1], F32, tag="zero")
                nc.vector.memset(zero_sb[:], 0.0)

            def h16_chunk(k, half):
                return h16_sb[:, half, k, :]

            # ---------------- index pipeline ----------------
            # logits.T [28, BL] fp16 inputs, fp32 PSUM. Bank-outer (batch
            # halves): bank 0's bits/idx/wrap/gathers launch while bank 1's
            # logits matmuls are still streaming.
            logit_ps = pp.tile([2 * NB, BL], F32, tag="acc", bufs=3)
            bits_sb = spool.tile([2 * NB, BL], BF16, tag="bits")
            idx16 = spool.tile([2, BL], I16, tag="idx16")
            stg_r = []
            for r in range(2):
                stg = spool.tile([32, 64], I16, tag="stage", name=f"stg{r}")
                stg_r.append(stg)
            # combined wrapped-idx tile: cols [n*64 + r*32 : +32] hold the
            # r-tensor indices of batch-half n
            idxw = spool.tile([128, 128], I16, tag="idxw")
            HB = BL // 2
            g2_tiles = [[None, None], [None, None]]

            for n in range(2):
                sl = slice(n * 512, (n + 1) * 512)
                for k in range(KC):
                    nc.tensor.matmul(
                        logit_ps[:, sl],
                        mw_sb[:, k, :],
                        h16_chunk(k, n),
                        start=(k == 0),
                        stop=(k == KC - 1),
                    )
                # bits = (h@Mw.T + Mb > 0)  <=>  (h@Mw.T > -Mb), as 1.0/0.0
                nc.vector.tensor_scalar(
                    bits_sb[:, sl], logit_ps[:, sl], negmb_sb[:, 0:1], None,
                    mybir.AluOpType.is_gt,
                )
                # raw indices via tiny matmul with powers of two: [2, 512]
                idx_ps = pp.tile([2, 512], F32, tag="sm", bufs=2)
                nc.tensor.matmul(
                    idx_ps[:], pw_sb, bits_sb[:, sl], start=True, stop=True
                )
                # clip + cast to int16; per-partition clip bounds:
                # row0 -> [0, 8191], row1 -> [8192, 16383]
                nc.vector.tensor_scalar(
                    idx16[:, sl], idx_ps[:],
                    clip_sb[:, 0:1], clip_sb[:, 1:2],
                    mybir.AluOpType.max, mybir.AluOpType.min,
                )
                # wrap this batch-half into the [16-group, 32] gather layout:
                # stage S[i, 32n+q'] = idx[(32n+i)*16+q'%16] via a strided
                # DMA (16 cols, duplicated), then 32x32 DVE transposes into
                # idxw columns [32n:32n+32]
                for r in range(2):
                    stg = stg_r[r]
                    stg_j = stg[0:32, :].rearrange("p (j hq) -> p j hq", j=2)
                    with nc.allow_non_contiguous_dma(
                        reason="tiny idx wrap staging"
                    ):
                        nc.scalar.dma_start(
                            stg[0:32, 32 * n : 32 * n + 16],
                            idx16[r : r + 1, sl].rearrange(
                                "p (a b) -> p a b", b=16
                            ),
                        )
                    nc.vector.tensor_copy(
                        stg_j[:, n, 16:32], stg_j[:, n, 0:16]
                    )
                    for g in range(4):
                        nc.vector.transpose(
                            idxw[32 * g : 32 * (g + 1),
                                 n * 64 + r * 32 : n * 64 + r * 32 + 32],
                            stg[:, 32 * n : 32 * n + 32],
                        )
                # one gather per (mem tensor, batch-half): smaller calls
                # put the first blocks in flight sooner (desc-gen ~7ns/row)
                for r in range(2):
                    g2 = gpool.tile(
                        [128, HB // 128, H], BF16, tag=f"g2_{r}{n}",
                        name=f"g2_{r}{n}",
                    )
                    nc.gpsimd.dma_gather(
                        out_ap=g2[:],
                        in_ap=mem_t[:],
                        idxs_ap=idxw[:, n * 64 + r * 32 : n * 64 + r * 32 + 32],
                        num_idxs=HB,
                        num_idxs_reg=HB,
                        elem_size=H,
                        transpose=False,
                    )
                    g2_tiles[r][n] = g2

            # ---------------- main matmuls + epilogue ----------------
            srcs_xh = [(x8_sb, 0), (h8_sb, 1)]
            ps_tiles = {}
            mem_sb = [[None] * MT, [None] * MT]

            def emit_transpose(c):
                # PE-transpose gathered rows of batch-block c into
                # [feat, batch] bf16 tiles; PSUM->SBUF copies alternate
                # DVE / ACT so neither engine rate-limits the pipeline
                for r in range(2):
                    g2 = g2_tiles[r][c // 4]
                    cc = c % 4
                    mt = gpool.tile([128, KC, 128], BF16, tag=f"mem{r}_{c}")
                    for k in range(KC):
                        tp = pp.tile([128, 128], BF16, tag="sm", bufs=2)
                        nc.tensor.transpose(
                            tp[:], g2[:, cc, k * 128 : (k + 1) * 128],
                            ident_sb[:],
                        )
                        if k % 2 == 0:
                            nc.vector.tensor_copy(mt[:, k, :], tp[:])
                        else:
                            nc.scalar.activation(
                                mt[:, k, :], tp[:],
                                mybir.ActivationFunctionType.Identity,
                            )
                    mem_sb[r][c] = mt

            def emit_xh(m):
                ps = pp.tile([128, H], F32, tag="acc", bufs=3)
                ps_tiles[m] = ps
                ms = slice(m * 128, (m + 1) * 128)
                # rank-1 bias matmul opens each bank's accumulation group
                for n in range(H // 512):
                    nc.tensor.matmul(
                        ps[:, n * 512 : (n + 1) * 512],
                        ones_sb[:],
                        bias16_sb[:, n * 512 : (n + 1) * 512],
                        start=True,
                        stop=False,
                    )
                # x/h terms in fp8 DoubleRow: two K-planes per
                # instruction, interleaved with the bf16 mem phases so the
                # activity governor stays quiet
                for si, (act, wi) in enumerate(srcs_xh):
                    for kp in range(KC // 2):
                        lhs = act[:, 2 * kp : 2 * kp + 2, ms]
                        for n in range(H // 512):
                            nc.tensor.matmul(
                                ps[:, n * 512 : (n + 1) * 512],
                                lhs,
                                w_sb[wi][:, 2 * kp : 2 * kp + 2,
                                         n * 512 : (n + 1) * 512],
                                start=False,
                                stop=False,
                                perf_mode=mybir.MatmulPerfMode.DoubleRow,
                            )

            def emit_mem_epilogue(m):
                ps = ps_tiles.pop(m)
                ms = slice(m * 128, (m + 1) * 128)
                st6 = epool.tile([128, 2, 6], F32, tag="st6")
                # bank-outer: bank 0 stops (and its bn_stats runs) while
                # bank 1's matmuls are still streaming
                for n in range(H // 512):
                    for si in range(2):
                        mt = mem_sb[si][m]  # [128, KC, 128] block for this m
                        for k in range(KC):
                            nc.tensor.matmul(
                                ps[:, n * 512 : (n + 1) * 512],
                                mt[:, k, :],
                                w_sb[2 + si][:, k, n * 512 : (n + 1) * 512],
                                start=False,
                                stop=(si == 1 and k == KC - 1),
                            )
                    nc.vector.bn_stats(
                        st6[:, n, :], ps[:, n * 512 : (n + 1) * 512]
                    )
                mv = epool.tile([128, 2], F32, tag="mv")
                nc.vector.bn_aggr(mv[:], st6.rearrange("p a b -> p (a b)"))
                # rstd = 1/sqrt(var+eps) entirely on DVE:
                # quake seed from vh=(var+eps)/2 bits, then 2 Newton steps
                # y <- y*(1.5 - vh*y^2).
                st = epool.tile([128, 4], F32, tag="rs")
                vh = st[:, 0:1]
                y = st[:, 1:2]
                a_ = st[:, 2:3]
                nmu = st[:, 3:4]
                nc.vector.tensor_scalar(
                    vh, mv[:, 1:2], 0.5, EPS_SC * 0.5,
                    mybir.AluOpType.mult, mybir.AluOpType.add,
                )
                nc.vector.tensor_scalar(
                    a_.bitcast(I32), vh.bitcast(I32), 1, None,
                    mybir.AluOpType.logical_shift_right,
                )
                nc.vector.tensor_tensor(
                    y.bitcast(I32), magic_sb[:], a_.bitcast(I32),
                    mybir.AluOpType.subtract,
                )
                for _ in range(1):
                    nc.vector.tensor_tensor(a_, y, y, mybir.AluOpType.mult)
                    nc.vector.tensor_tensor(a_, a_, vh, mybir.AluOpType.mult)
                    nc.vector.tensor_scalar(
                        a_, a_, 1.5, -1.0,
                        mybir.AluOpType.subtract, mybir.AluOpType.mult,
                    )
                    nc.vector.tensor_tensor(y, y, a_, mybir.AluOpType.mult)
                nc.vector.tensor_scalar(
                    nmu, mv[:, 0:1], y, -1.0,
                    mybir.AluOpType.mult, mybir.AluOpType.mult,
                )
                o = epool.tile([128, H], BF16, tag="o")
                if trivial_gb:
                    # out = sigmoid((t - mu) * rstd), read from PSUM
                    nc.scalar.activation(
                        o[:], ps[:], mybir.ActivationFunctionType.Sigmoid,
                        bias=nmu, scale=y,
                    )
                else:
                    xh = epool.tile([128, H], F32, tag="xh")
                    nc.scalar.activation(
                        xh[:], ps[:], mybir.ActivationFunctionType.Identity,
                        bias=nmu, scale=y,
                    )
                    nc.vector.tensor_tensor(
                        xh[:], xh[:], gam_sb[:], mybir.AluOpType.mult
                    )
                    nc.vector.tensor_tensor(
                        xh[:], xh[:], bet_sb[:], mybir.AluOpType.add
                    )
                    nc.scalar.activation(
                        o[:], xh[:], mybir.ActivationFunctionType.Sigmoid,
                        bias=zero_sb[:, 0:1],
                    )
                nc.sync.dma_start(out_t[ms, :], o[:])

            # "acc" slot ring: L(s0) x0(s1) x1(s2) x2(s0) x3(s1) x4(s2)
            # x5(s0) x6(s1) x7(s2); each reuse is covered by >=1 emitted
            # mem-tile of PE work so the epilogue chain never stalls the PE.
            emit_xh(0)
            emit_xh(1)
            emit_xh(2)
            for c in range(4):
                emit_transpose(c)
            emit_mem_epilogue(0)
            emit_mem_epilogue(1)
            emit_xh(3)
            emit_xh(4)
            for c in range(4, 8):
                emit_transpose(c)
            emit_mem_epilogue(2)
            emit_mem_epilogue(3)
            emit_xh(5)
            emit_xh(6)
            emit_mem_epilogue(4)
            emit_mem_epilogue(5)
            emit_xh(7)
            emit_mem_epilogue(6)
            emit_mem_epilogue(7)

    nc.compile()  # bacc register allocation / DCE
    return nc


def _to_kxp(a, dtype):
    """[batch, feat] -> [128, KC, batch] with feat = k*128 + p."""
    t = np.ascontiguousarray(a.T.reshape(KC, 128, -1).transpose(1, 0, 2))
    return t.astype(dtype)


def prep(inputs):
    """Host-side shard/layout prep. Returns (in_maps, trivial_gb)."""
    x = np.asarray(inputs["x"], np.float32)
    h = np.asarray(inputs["h_prev"], np.float32)
    memory = np.asarray(inputs["memory"], np.float32)
    gamma = np.asarray(inputs["gamma"], np.float32)
    beta = np.asarray(inputs["beta"], np.float32)
    trivial_gb = bool(np.all(gamma == 1.0) and np.all(beta == 0.0))

    bf = ml_dtypes.bfloat16
    e4 = ml_dtypes.float8_e4m3
    # W is [out, in]; the kernel wants w[p, k, n] = W[n, k*128+p], which is
    # exactly _to_kxp applied to W with (out, in) in the (batch, feat) slots.
    w8_cat = np.stack(
        [
            _to_kxp(np.asarray(inputs[n], np.float32) * WSCALE, e4)
            for n in ("Ww", "Uw")
        ]
    )
    wq_cat = np.stack(
        [
            _to_kxp(np.asarray(inputs[n], np.float32) * WSCALE, bf)
            for n in ("Qrw", "Qlw")
        ]
    )
    mw = _to_kxp(np.asarray(inputs["Mw"], np.float32), np.float16)

    pw = np.zeros((2 * NB, 2), np.float32)
    pw[:NB, 0] = 2.0 ** np.arange(NB - 1, -1, -1)
    pw[NB:, 1] = 2.0 ** np.arange(NB - 1, -1, -1)
    clip = np.array(
        [[0.0, MEM // 2 - 1], [MEM // 2, MEM - 1]], np.float32
    )  # [row, (lo, hi)]

    mem16 = memory.astype(bf)
    bias16 = (
        (
            np.asarray(inputs["Wb"], np.float32)
            + np.asarray(inputs["Ub"], np.float32)
            + np.asarray(inputs["Qrb"], np.float32)
            + np.asarray(inputs["Qlb"], np.float32)
        )
        * WSCALE
    ).astype(bf).reshape(1, H)

    # pack the small constants into one [128, NCONST] f32 buffer
    const = np.zeros((128, NCONST), np.float32)
    const[:2, C_CLIP : C_CLIP + 2] = clip
    const[: 2 * NB, C_NEGMB : C_NEGMB + 1] = -np.asarray(
        inputs["Mb"], np.float32
    ).reshape(2 * NB, 1)
    const[: 2 * NB, C_PW : C_PW + 1] = pw.astype(bf).view(np.float32)
    const[:, C_MAGIC : C_MAGIC + 1] = (
        np.full((128, 1), RSQRT_MAGIC, np.int32).view(np.float32)
    )
    ident16 = np.eye(128, dtype=np.float32).astype(bf)
    const[:, C_IDENT : C_IDENT + 64] = ident16.view(np.float32)

    common = dict(
        w8_t=w8_cat, wq_t=wq_cat, bias16_t=bias16, const_t=const,
        mem_t=mem16, mw_t=mw,
    )
    if not trivial_gb:
        common["gam_t"] = np.ascontiguousarray(np.broadcast_to(gamma, (128, H)))
        common["bet_t"] = np.ascontiguousarray(np.broadcast_to(beta, (128, H)))

    in_maps = []
    for c in range(NCORES):
        xs = x[c * BL : (c + 1) * BL]
        hs = h[c * BL : (c + 1) * BL]
        in_maps.append(
            dict(
                x8_t=_to_kxp(xs, e4),
                h8_t=_to_kxp(hs, e4),
                h16_t=np.ascontiguousarray(
                    _to_kxp(hs, np.float16).reshape(128, KC, 2, 512)
                    .transpose(0, 2, 1, 3)
                ),
                **common,
            )
        )
    return in_maps, trivial_gb


def get_nc(trivial_gb):
    key = ("nc", trivial_gb)
    if key not in _CACHE:
        _CACHE[key] = _build(trivial_gb)
    return _CACHE[key]


def run(inputs, trace=False, **kw):
    in_maps, trivial_gb = prep(inputs)
    nc = get_nc(trivial_gb)
    res = run_bass_kernel_spmd(
        nc, in_maps, core_ids=list(range(NCORES)), trace=trace, **kw
    )
    out = np.concatenate([res.results[c]["out_t"] for c in range(NCORES)], axis=0)
    return out.astype(np.float32), res


def kernel(**inputs):
    return run(inputs)[0]


# revision 53
# speedup vs baseline: 1.6703x; 1.1898x over previous
"""Trainium2 Bass kernel for nn_BinaryMemoryRNN (scatter_memory).

Computation (reference):
    logits = h_prev @ Mw.T + Mb                 # [B, 28]
    b1/b2  = bits of logits halves (> 0)
    idx1   = clip(sum(b1 * 2^(13-j)), 0, 8191)
    idx2   = clip(sum(b2 * 2^(13-j)), 8192, 16383)
    pre    = x @ Ww.T + h_prev @ Uw.T + mem[idx1] @ Qrw.T + mem[idx2] @ Qlw.T + bias
    out    = sigmoid(layernorm(pre) * gamma + beta)

Strategy: data-parallel over batch across 8 cores (1024 rows each).
  - x/h activations in fp8-e4m3 as the stationary matmul operand; weights
    stream as bf16 (the fp8xfp8 combination runs at half rate on TRN2, and
    fp8 as the moving operand is slow - fp8-stationary x bf16-moving is
    full rate). Weights scaled x256 so fp8 activations meet them in a
    range the layernorm renormalizes away; only the bias needs the x256.
  - logits matmul in fp16 (sign-sensitive index bits: fp16's 10 mantissa
    bits flip ~1e-3 of them; full fp32 would cost 4x the PE time and 2x
    the critical-path DMA).
  - memory table replicated in DRAM as bf16 [16384, 1024]; rows fetched
    with gpsimd.dma_gather (row layout), PE-transposed to [feat, batch].
    PSUM->SBUF copies of transposed tiles alternate DVE / ACT engines.
  - bias enters the PSUM accumulation as a rank-1 (ones x bias) matmul,
    so the epilogue reads layernorm stats straight from PSUM; mem matmuls
    are emitted bank-outer so bank-0 stats overlap bank-1 matmuls.
  - rstd = 1/sqrt(var+eps) via DVE quake-seed + 2 Newton steps: the ACT
    engine's activation table never leaves Sigmoid. Output written bf16.
  - DMA issue order doubles as the priority schedule: logits path first
    (mw, h16 pieces), then x/w0 for the first matmuls, then the rest.
"""

import sys

sys.path.insert(0, "/opt/trn_rl_repo")

from contextlib import ExitStack

import numpy as np
import ml_dtypes

import concourse.bass as bass
import concourse.tile as tile
from concourse import bacc, mybir, library_config
from concourse.bass_utils import run_bass_kernel_spmd

F32 = mybir.dt.float32
F16 = mybir.dt.float16
BF16 = mybir.dt.bfloat16
F8E4 = mybir.dt.float8e4
I16 = mybir.dt.int16
I32 = mybir.dt.int32

B, I, H, NB = 8192, 1024, 1024, 14
MEM = 2**NB
NCORES = 8
BL = B // NCORES  # 1024 batch rows per core
KC = H // 128  # 8 contraction chunks
MT = BL // 128  # 8 output row-tiles per core
EPS = 1e-5
WSCALE = 256.0
EPS_SC = EPS * WSCALE * WSCALE  # eps for the x256-scaled pre-activation
RSQRT_MAGIC = 0x5EF759DF  # 0x5f3759df - 0x00400000: seed for rsqrt(2*vh)

# const_t packed layout (f32 columns)
C_CLIP = 0  # [2, 2] idx clip bounds
C_NEGMB = 2  # [28, 1] -Mb
C_PW = 3  # [28, 1] powers of two as bf16 pair-packed in f32
C_MAGIC = 4  # [128, 1] rsqrt seed magic (int32 bits)
C_IDENT = 5  # [128, 64] 128x128 bf16 identity (bitcast)
NCONST = 69

_CACHE = {}


def _build(trivial_gb: bool):
    """Trace the Bass/Tile module (shared by all 8 cores, SPMD)."""
    nc = bacc.Bacc(
        "TRN2", target_bir_lowering=False, debug=False, enable_asserts=True
    )

    x8_t = nc.dram_tensor("x8_t", [128, KC, BL], F8E4, kind="ExternalInput").ap()
    h8_t = nc.dram_tensor("h8_t", [128, KC, BL], F8E4, kind="ExternalInput").ap()
    h16_t = nc.dram_tensor("h16_t", [128, 2, KC, 512], F16, kind="ExternalInput").ap()
    mw_t = nc.dram_tensor("mw_t", [128, KC, 2 * NB], F16, kind="ExternalInput").ap()
    # weights, [src, feat_in(part), feat_in(chunk), feat_out]; all fp8
    w8_t = nc.dram_tensor("w8_t", [4, 128, KC, H], F8E4, kind="ExternalInput").ap()
    bias16_t = nc.dram_tensor("bias16_t", [1, H], BF16, kind="ExternalInput").ap()
    const_t = nc.dram_tensor("const_t", [128, NCONST], F32, kind="ExternalInput").ap()
    mem_t = nc.dram_tensor("mem_t", [MEM, H], BF16, kind="ExternalInput").ap()
    if not trivial_gb:
        gam_t = nc.dram_tensor("gam_t", [128, H], F32, kind="ExternalInput").ap()
        bet_t = nc.dram_tensor("bet_t", [128, H], F32, kind="ExternalInput").ap()
    out_t = nc.dram_tensor("out_t", [BL, H], BF16, kind="ExternalOutput").ap()

    with tile.TileContext(nc) as tc:
        with ExitStack() as ctx:
            # ---------------- pools ----------------
            cpool = ctx.enter_context(tc.tile_pool(name="consts", bufs=1))
            apool = ctx.enter_context(tc.tile_pool(name="acts", bufs=1))
            hpool = ctx.enter_context(tc.tile_pool(name="h16", bufs=1))
            gpool = ctx.enter_context(tc.tile_pool(name="gathered", bufs=1))
            spool = ctx.enter_context(tc.tile_pool(name="small", bufs=2))
            epool = ctx.enter_context(tc.tile_pool(name="epilogue", bufs=2))
            # PSUM: tag "acc" rotates 3 two-bank slots (logits shares the
            # ring); tag "sm" rotates 2 one-bank slots (idx halves + PE
            # transposes). 3*2 + 2*1 = 8 banks.
            pp = ctx.enter_context(tc.tile_pool(name="psum", bufs=1, space="PSUM"))

            # gpsimd ucode library containing DMAGatherAnt; load it up front
            # so the Q7 IRAM reload overlaps the initial DMAs.
            nc.gpsimd.load_library(library_config.attnmlp)

            # ---------------- input loads (issue order = priority) ----------
            mw_sb = cpool.tile([128, KC, 2 * NB], F16, tag="mw")
            nc.sync.dma_start(mw_sb[:], mw_t[:])
            const_sb = cpool.tile([128, NCONST], F32, tag="const")
            nc.sync.dma_start(const_sb[:], const_t[:])
            clip_sb = const_sb[0:2, C_CLIP : C_CLIP + 2]
            negmb_sb = const_sb[0 : 2 * NB, C_NEGMB : C_NEGMB + 1]
            pw_sb = const_sb[0 : 2 * NB, C_PW : C_PW + 1].bitcast(BF16)
            magic_sb = const_sb[:, C_MAGIC : C_MAGIC + 1].bitcast(I32)
            ident_sb = const_sb[:, C_IDENT : C_IDENT + 64].bitcast(BF16)

            # warm the Sigmoid activation table while DMAs run (the only
            # ACT table in this kernel -> one table load total)
            warm = cpool.tile([128, 1], F32, tag="warm")
            nc.vector.memset(warm[:], 0.0)
            nc.scalar.activation(
                warm[:], warm[:], mybir.ActivationFunctionType.Sigmoid
            )
            ones_sb = cpool.tile([1, 128], BF16, tag="ones")
            nc.vector.memset(ones_sb[:], 1.0)

            # h16 for logits, half-major [128, half, KC, 512] so each
            # batch-half is one contiguous DMA and bank 0's logits/idx/
            # gathers launch after 1.1MB
            h16_sb = hpool.tile([128, 2, KC, 512], F16, tag="h16")
            x8_sb = apool.tile([128, KC, BL], F8E4, tag="x8")
            h8_sb = apool.tile([128, KC, BL], F8E4, tag="h8")
            w_sb = [
                cpool.tile([128, KC, H], F8E4, tag=f"w{s}", name=f"w{s}")
                for s in range(4)
            ]
            bias16_sb = cpool.tile([1, H], BF16, tag="bias16")

            for n in range(2):
                nc.sync.dma_start(h16_sb[:, n], h16_t[:, n])
            nc.sync.dma_start(bias16_sb[:], bias16_t[:])
            # x and weights follow in k-chunk pieces so the xh matmuls can
            # start on partial data right after logits
            nc.sync.dma_start(x8_sb[:, 0:4, :], x8_t[:, 0:4, :])
            nc.sync.dma_start(x8_sb[:, 4:8, :], x8_t[:, 4:8, :])
            for kk in range(2):
                nc.sync.dma_start(
                    w_sb[0][:, 4 * kk : 4 * kk + 4, :],
                    w8_t[0, :, 4 * kk : 4 * kk + 4, :],
                )
            nc.sync.dma_start(h8_sb[:, 0:4, :], h8_t[:, 0:4, :])
            nc.sync.dma_start(h8_sb[:, 4:8, :], h8_t[:, 4:8, :])
            for kk in range(2):
                nc.sync.dma_start(
                    w_sb[1][:, 4 * kk : 4 * kk + 4, :],
                    w8_t[1, :, 4 * kk : 4 * kk + 4, :],
                )
            for s in (2, 3):
                for kk in range(2):
                    nc.sync.dma_start(
                        w_sb[s][:, 4 * kk : 4 * kk + 4, :],
                        w8_t[s, :, 4 * kk : 4 * kk + 4, :],
                    )
            if not trivial_gb:
                gam_sb = cpool.tile([128, H], F32, tag="gam")
                nc.sync.dma_start(gam_sb[:], gam_t[:])
                bet_sb = cpool.tile([128, H], F32, tag="bet")
                nc.sync.dma_start(bet_sb[:], bet_t[:])
                zero_sb = cpool.tile([128, 1], F32, tag="zero")
                nc.vector.memset(zero_sb[:], 0.0)

            def h16_chunk(k, half):
                return h16_sb[:, half, k, :]

            # ---------------- index pipeline ----------------
            # logits.T [28, BL] fp16 inputs, fp32 PSUM. Bank-outer (batch
            # halves): bank 0's bits/idx/wrap/gathers launch while bank 1's
            # logits matmuls are still streaming.
            logit_ps = pp.tile([2 * NB, BL], F32, tag="acc", bufs=3)
            bits_sb = spool.tile([2 * NB, BL], BF16, tag="bits")
            idx16 = spool.tile([2, BL], I16, tag="idx16")
            stg_r = []
            for r in range(2):
                stg = spool.tile([32, 64], I16, tag="stage", name=f"stg{r}")
                stg_r.append(stg)
            # combined wrapped-idx tile: cols [n*64 + r*32 : +32] hold the
            # r-tensor indices of batch-half n
            idxw = spool.tile([128, 128], I16, tag="idxw")
            HB = BL // 2
            g2_tiles = [[None, None], [None, None]]

            for n in range(2):
                sl = slice(n * 512, (n + 1) * 512)
                for k in range(KC):
                    nc.tensor.matmul(
                        logit_ps[:, sl],
                        mw_sb[:, k, :],
                        h16_chunk(k, n),
                        start=(k == 0),
                        stop=(k == KC - 1),
                    )
                # bits = (h@Mw.T + Mb > 0)  <=>  (h@Mw.T > -Mb), as 1.0/0.0
                nc.vector.tensor_scalar(
                    bits_sb[:, sl], logit_ps[:, sl], negmb_sb[:, 0:1], None,
                    mybir.AluOpType.is_gt,
                )
                # raw indices via tiny matmul with powers of two: [2, 512]
                idx_ps = pp.tile([2, 512], F32, tag="sm", bufs=2)
                nc.tensor.matmul(
                    idx_ps[:], pw_sb, bits_sb[:, sl], start=True, stop=True
                )
                # clip + cast to int16; per-partition clip bounds:
                # row0 -> [0, 8191], row1 -> [8192, 16383]
                nc.vector.tensor_scalar(
                    idx16[:, sl], idx_ps[:],
                    clip_sb[:, 0:1], clip_sb[:, 1:2],
                    mybir.AluOpType.max, mybir.AluOpType.min,
                )
                # wrap this batch-half into the [16-group, 32] gather layout:
                # stage S[i, 32n+q'] = idx[(32n+i)*16+q'%16] via a strided
                # DMA (16 cols, duplicated), then 32x32 DVE transposes into
                # idxw columns [32n:32n+32]
                for r in range(2):
                    stg = stg_r[r]
                    stg_j = stg[0:32, :].rearrange("p (j hq) -> p j hq", j=2)
                    with nc.allow_non_contiguous_dma(
                        reason="tiny idx wrap staging"
                    ):
                        nc.scalar.dma_start(
                            stg[0:32, 32 * n : 32 * n + 16],
                            idx16[r : r + 1, sl].rearrange(
                                "p (a b) -> p a b", b=16
                            ),
                        )
                    nc.vector.tensor_copy(
                        stg_j[:, n, 16:32], stg_j[:, n, 0:16]
                    )
                    for g in range(4):
                        nc.vector.transpose(
                            idxw[32 * g : 32 * (g + 1),
                                 n * 64 + r * 32 : n * 64 + r * 32 + 32],
                            stg[:, 32 * n : 32 * n + 32],
                        )
                # one gather per (mem tensor, batch-half): smaller calls
                # put the first blocks in flight sooner (desc-gen ~7ns/row)
                for r in range(2):
                    g2 = gpool.tile(
                        [128, HB // 128, H], BF16, tag=f"g2_{r}{n}",
                        name=f"g2_{r}{n}",
                    )
                    nc.gpsimd.dma_gather(
                        out_ap=g2[:],
                        in_ap=mem_t[:],
                        idxs_ap=idxw[:, n * 64 + r * 32 : n * 64 + r * 32 + 32],
                        num_idxs=HB,
                        num_idxs_reg=HB,
                        elem_size=H,
                        transpose=False,
                    )
                    g2_tiles[r][n] = g2

            # ---------------- main matmuls + epilogue ----------------
            srcs_xh = [(x8_sb, 0), (h8_sb, 1)]
            ps_tiles = {}
            mem_sb = [[None] * MT, [None] * MT]

            def emit_transpose(c):
                # PE-transpose gathered rows of batch-block c into
                # [feat, batch] bf16 tiles; PSUM->SBUF copies alternate
                # DVE / ACT so neither engine rate-limits the pipeline
                for r in range(2):
                    g2 = g2_tiles[r][c // 4]
                    cc = c % 4
                    mt = gpool.tile([128, KC, 128], F8E4, tag=f"mem{r}_{c}")
                    for k in range(KC):
                        tp = pp.tile([128, 128], BF16, tag="sm", bufs=2)
                        nc.tensor.transpose(
                            tp[:], g2[:, cc, k * 128 : (k + 1) * 128],
                            ident_sb[:],
                        )
                        if k % 2 == 0:
                            nc.vector.tensor_copy(mt[:, k, :], tp[:])
                        else:
                            nc.scalar.activation(
                                mt[:, k, :], tp[:],
                                mybir.ActivationFunctionType.Identity,
                            )
                    mem_sb[r][c] = mt

            def emit_xh(m):
                ps = pp.tile([128, H], F32, tag="acc", bufs=3)
                ps_tiles[m] = ps
                ms = slice(m * 128, (m + 1) * 128)
                # rank-1 bias matmul opens each bank's accumulation group
                for n in range(H // 512):
                    nc.tensor.matmul(
                        ps[:, n * 512 : (n + 1) * 512],
                        ones_sb[:],
                        bias16_sb[:, n * 512 : (n + 1) * 512],
                        start=True,
                        stop=False,
                    )
                # x/h terms in fp8 DoubleRow: two K-planes per
                # instruction, interleaved with the bf16 mem phases so the
                # activity governor stays quiet
                for si, (act, wi) in enumerate(srcs_xh):
                    for kp in range(KC // 2):
                        lhs = act[:, 2 * kp : 2 * kp + 2, ms]
                        for n in range(H // 512):
                            nc.tensor.matmul(
                                ps[:, n * 512 : (n + 1) * 512],
                                lhs,
                                w_sb[wi][:, 2 * kp : 2 * kp + 2,
                                         n * 512 : (n + 1) * 512],
                                start=False,
                                stop=False,
                                perf_mode=mybir.MatmulPerfMode.DoubleRow,
                            )

            def emit_mem_epilogue(m):
                ps = ps_tiles.pop(m)
                ms = slice(m * 128, (m + 1) * 128)
                st6 = epool.tile([128, 2, 6], F32, tag="st6")
                # bank-outer: bank 0 stops (and its bn_stats runs) while
                # bank 1's matmuls are still streaming
                for n in range(H // 512):
                    for si in range(2):
                        mt = mem_sb[si][m]  # [128, KC, 128] block for this m
                        for kp in range(KC // 2):
                            nc.tensor.matmul(
                                ps[:, n * 512 : (n + 1) * 512],
                                mt[:, 2 * kp : 2 * kp + 2, :],
                                w_sb[2 + si][:, 2 * kp : 2 * kp + 2,
                                             n * 512 : (n + 1) * 512],
                                start=False,
                                stop=(si == 1 and kp == KC // 2 - 1),
                                perf_mode=mybir.MatmulPerfMode.DoubleRow,
                            )
                    nc.vector.bn_stats(
                        st6[:, n, :], ps[:, n * 512 : (n + 1) * 512]
                    )
                mv = epool.tile([128, 2], F32, tag="mv")
                nc.vector.bn_aggr(mv[:], st6.rearrange("p a b -> p (a b)"))
                # rstd = 1/sqrt(var+eps) entirely on DVE:
                # quake seed from vh=(var+eps)/2 bits, then 2 Newton steps
                # y <- y*(1.5 - vh*y^2).
                st = epool.tile([128, 4], F32, tag="rs")
                vh = st[:, 0:1]
                y = st[:, 1:2]
                a_ = st[:, 2:3]
                nmu = st[:, 3:4]
                nc.vector.tensor_scalar(
                    vh, mv[:, 1:2], 0.5, EPS_SC * 0.5,
                    mybir.AluOpType.mult, mybir.AluOpType.add,
                )
                nc.vector.tensor_scalar(
                    a_.bitcast(I32), vh.bitcast(I32), 1, None,
                    mybir.AluOpType.logical_shift_right,
                )
                nc.vector.tensor_tensor(
                    y.bitcast(I32), magic_sb[:], a_.bitcast(I32),
                    mybir.AluOpType.subtract,
                )
                for _ in range(1):
                    nc.vector.tensor_tensor(a_, y, y, mybir.AluOpType.mult)
                    nc.vector.tensor_tensor(a_, a_, vh, mybir.AluOpType.mult)
                    nc.vector.tensor_scalar(
                        a_, a_, 1.5, -1.0,
                        mybir.AluOpType.subtract, mybir.AluOpType.mult,
                    )
                    nc.vector.tensor_tensor(y, y, a_, mybir.AluOpType.mult)
                nc.vector.tensor_scalar(
                    nmu, mv[:, 0:1], y, -1.0,
                    mybir.AluOpType.mult, mybir.AluOpType.mult,
                )
                o = epool.tile([128, H], BF16, tag="o")
                if trivial_gb:
                    # out = sigmoid((t - mu) * rstd), read from PSUM
                    nc.scalar.activation(
                        o[:], ps[:], mybir.ActivationFunctionType.Sigmoid,
                        bias=nmu, scale=y,
                    )
                else:
                    xh = epool.tile([128, H], F32, tag="xh")
                    nc.scalar.activation(
                        xh[:], ps[:], mybir.ActivationFunctionType.Identity,
                        bias=nmu, scale=y,
                    )
                    nc.vector.tensor_tensor(
                        xh[:], xh[:], gam_sb[:], mybir.AluOpType.mult
                    )
                    nc.vector.tensor_tensor(
                        xh[:], xh[:], bet_sb[:], mybir.AluOpType.add
                    )
                    nc.scalar.activation(
                        o[:], xh[:], mybir.ActivationFunctionType.Sigmoid,
                        bias=zero_sb[:, 0:1],
                    )
                nc.sync.dma_start(out_t[ms, :], o[:])

            # "acc" slot ring: L(s0) x0(s1) x1(s2) x2(s0) x3(s1) x4(s2)
            # x5(s0) x6(s1) x7(s2); each reuse is covered by >=1 emitted
            # mem-tile of PE work so the epilogue chain never stalls the PE.
            emit_xh(0)
            emit_xh(1)
            emit_xh(2)
            for c in range(4):
                emit_transpose(c)
            emit_mem_epilogue(0)
            emit_mem_epilogue(1)
            emit_xh(3)
            emit_xh(4)
            for c in range(4, 8):
                emit_transpose(c)
            emit_mem_epilogue(2)
            emit_mem_epilogue(3)
            emit_xh(5)
            emit_xh(6)
            emit_mem_epilogue(4)
            emit_mem_epilogue(5)
            emit_xh(7)
            emit_mem_epilogue(6)
            emit_mem_epilogue(7)

    nc.compile()  # bacc register allocation / DCE
    return nc


def _to_kxp(a, dtype):
    """[batch, feat] -> [128, KC, batch] with feat = k*128 + p."""
    t = np.ascontiguousarray(a.T.reshape(KC, 128, -1).transpose(1, 0, 2))
    return t.astype(dtype)


def prep(inputs):
    """Host-side shard/layout prep. Returns (in_maps, trivial_gb)."""
    x = np.asarray(inputs["x"], np.float32)
    h = np.asarray(inputs["h_prev"], np.float32)
    memory = np.asarray(inputs["memory"], np.float32)
    gamma = np.asarray(inputs["gamma"], np.float32)
    beta = np.asarray(inputs["beta"], np.float32)
    trivial_gb = bool(np.all(gamma == 1.0) and np.all(beta == 0.0))

    bf = ml_dtypes.bfloat16
    e4 = ml_dtypes.float8_e4m3
    # W is [out, in]; the kernel wants w[p, k, n] = W[n, k*128+p], which is
    # exactly _to_kxp applied to W with (out, in) in the (batch, feat) slots.
    w8_cat = np.stack(
        [
            _to_kxp(np.asarray(inputs[n], np.float32) * WSCALE, e4)
            for n in ("Ww", "Uw", "Qrw", "Qlw")
        ]
    )
    mw = _to_kxp(np.asarray(inputs["Mw"], np.float32), np.float16)

    pw = np.zeros((2 * NB, 2), np.float32)
    pw[:NB, 0] = 2.0 ** np.arange(NB - 1, -1, -1)
    pw[NB:, 1] = 2.0 ** np.arange(NB - 1, -1, -1)
    clip = np.array(
        [[0.0, MEM // 2 - 1], [MEM // 2, MEM - 1]], np.float32
    )  # [row, (lo, hi)]

    mem16 = memory.astype(bf)
    bias16 = (
        (
            np.asarray(inputs["Wb"], np.float32)
            + np.asarray(inputs["Ub"], np.float32)
            + np.asarray(inputs["Qrb"], np.float32)
            + np.asarray(inputs["Qlb"], np.float32)
        )
        * WSCALE
    ).astype(bf).reshape(1, H)

    # pack the small constants into one [128, NCONST] f32 buffer
    const = np.zeros((128, NCONST), np.float32)
    const[:2, C_CLIP : C_CLIP + 2] = clip
    const[: 2 * NB, C_NEGMB : C_NEGMB + 1] = -np.asarray(
        inputs["Mb"], np.float32
    ).reshape(2 * NB, 1)
    const[: 2 * NB, C_PW : C_PW + 1] = pw.astype(bf).view(np.float32)
    const[:, C_MAGIC : C_MAGIC + 1] = (
        np.full((128, 1), RSQRT_MAGIC, np.int32).view(np.float32)
    )
    ident16 = np.eye(128, dtype=np.float32).astype(bf)
    const[:, C_IDENT : C_IDENT + 64] = ident16.view(np.float32)

    common = dict(
        w8_t=w8_cat, bias16_t=bias16, const_t=const, mem_t=mem16, mw_t=mw,
    )
    if not trivial_gb:
        common["gam_t"] = np.ascontiguousarray(np.broadcast_to(gamma, (128, H)))
        common["bet_t"] = np.ascontiguousarray(np.broadcast_to(beta, (128, H)))

    in_maps = []
    for c in range(NCORES):
        xs = x[c * BL : (c + 1) * BL]
        hs = h[c * BL : (c + 1) * BL]
        in_maps.append(
            dict(
                x8_t=_to_kxp(xs, e4),
                h8_t=_to_kxp(hs, e4),
                h16_t=np.ascontiguousarray(
                    _to_kxp(hs, np.float16).reshape(128, KC, 2, 512)
                    .transpose(0, 2, 1, 3)
                ),
                **common,
            )
        )
    return in_maps, trivial_gb


def get_nc(trivial_gb):
    key = ("nc", trivial_gb)
    if key not in _CACHE:
        _CACHE[key] = _build(trivial_gb)
    return _CACHE[key]


def run(inputs, trace=False, **kw):
    in_maps, trivial_gb = prep(inputs)
    nc = get_nc(trivial_gb)
    res = run_bass_kernel_spmd(
        nc, in_maps, core_ids=list(range(NCORES)), trace=trace, **kw
    )
    out = np.concatenate([res.results[c]["out_t"] for c in range(NCORES)], axis=0)
    return out.astype(np.float32), res


def kernel(**inputs):
    return run(inputs)[0]
